# revision 16
# baseline (speedup 1.0000x reference)
"""Trainium2 Bass kernel for nn_AttentionModule_50002009260608.

B=16, C=512, H=W=24 (HW=576), TF=512, NH=8, CPH=64.
Data-parallel over batch: 2 batch elements per core x 8 cores.
Weights replicated; host pre-transposes 1x1-conv weights to [c_in, c_out]
and precomputes the two tiny text matvecs (t_m, Wm2 @ t).

All heavy matmuls run as float32r (full PE rate for N>=256) accumulating
in fp32 PSUM. fp32r ISA restrictions: output must span all 128 PE columns
(M>=97) and innermost AP counts must be even -- hence hw m-tiles of
116+115*4 and the padded per-head V'T stride of 128.
"""

import numpy as np
from contextlib import ExitStack

import concourse.bacc as bacc
import concourse.bass as bass
import concourse.tile as tile
import concourse.mybir as mybir
from concourse import masks
from concourse.bass_utils import run_bass_kernel_spmd

B, C, HW, TF, NH, CPH = 16, 512, 576, 512, 8, 64
NCORES, BPC = 8, B // 8
SCALE = 1.0 / 8.0  # 1/sqrt(CPH)
F32, F32R = mybir.dt.float32, mybir.dt.float32r
AF = mybir.ActivationFunctionType
OP = mybir.AluOpType
PD = 128
NCC = C // PD                                    # 4 channel chunks
MT = [(0, 116), (116, 115), (231, 115), (346, 115), (461, 115)]  # hw m-tiles
NHALF = [(0, 288), (288, 288)]                   # softmax eviction halves
AVCH = [(0, 288), (288, 290)]                    # AV rhs chunks over es cols
CPS = 128                                        # padded per-head V'T stride
TMP = 104                                        # padded t_m_blk cols (fp32r M>=97)
ESW = HW + 2                                     # es cols: 576 + cross col + pad


def _body(ctx: ExitStack, tc, d):
    """d: DRAM APs: x[2,512,576](f32r), t_m_blk[2,512,104](f32r),
    tvec[2,512,1], WqT/WkT/WvT/Wm1T/WrT [512,512](f32r, pre-transposed
    [c_in,c_out]), Wr_b[512,1], out[2,512,576]."""
    nc = tc.nc

    wt = ctx.enter_context(tc.tile_pool(name="wt", bufs=1))
    act = ctx.enter_context(tc.tile_pool(name="act", bufs=1))
    expp = ctx.enter_context(tc.tile_pool(name="expp", bufs=1))
    ps = ctx.enter_context(tc.tile_pool(name="ps", bufs=1, space="PSUM"))

    # ---- batch-0 activations first (PE can start within ~2us), then weights,
    # all split per channel-chunk so the first conv group's deps arrive early ----
    xbts = []
    for b in range(BPC):
        xbt = act.tile([PD, NCC * HW], F32R, name=f"xb{b}", tag="xb", bufs=2)
        if b == 0:
            for j in range(NCC):
                nc.sync.dma_start(xbt[:, j * HW:(j + 1) * HW],
                                  d["x"][b, j * PD:(j + 1) * PD, :])
        xbts.append(xbt)
    W = {}
    for wn in ("WqT", "WkT", "Wm1T", "WvT", "WrT"):
        wtile = wt.tile([PD, NCC * C], F32R, name=f"{wn}_t")
        for j in range(NCC):
            nc.sync.dma_start(wtile[:, j * C:(j + 1) * C],
                              d[wn][j * PD:(j + 1) * PD, :])
        W[wn] = [wtile[:, j * C:(j + 1) * C] for j in range(NCC)]
    wrbt = wt.tile([PD, NCC], F32, name="wrbt")
    nc.sync.dma_start(wrbt[:], d["Wr_b"].rearrange("(cc p) one -> p (cc one)", p=PD))
    wrb = [wrbt[:, j:j + 1] for j in range(NCC)]
    ident = wt.tile([PD, PD], F32, name="ident")
    masks.make_identity(nc, ident[:])
    onesb = wt.tile([PD, (CPS - CPH) * NH], F32, name="onesb")
    nc.vector.memset(onesb[:], 1.0)
    # persistent V'T tiles: [hw_tile, 8*128]; per head block: cols 0:64 = V_h^T,
    # cols 64:128 = 1.0 (fused softmax column sums). Ones written once.
    VT = [wt.tile([sz, NH * CPS], F32R, name=f"vt{mi}")
          for mi, (m0, sz) in enumerate(MT)]
    for mi, (m0, sz) in enumerate(MT):
        nc.vector.tensor_copy(
            VT[mi][:].rearrange("p (h c) -> p h c", h=NH)[:, :, CPH:CPS],
            onesb[0:sz, :])

    def conv(name, b, Wn, rhs, outs, bias=None):
        # outs[ot][:, n] = sum_cc Wn[cc][:, ot*128:+128].T @ rhs[cc][:, n] (+ bias)
        for ot in range(NCC):
            for (n0, nsz) in NHALF:
                p = ps.tile([PD, nsz], F32, tag="conv", bufs=2,
                            name=f"p_{name}{b}_{ot}_{n0}")
                for cc in range(NCC):
                    nc.tensor.matmul(
                        p[:], Wn[cc][:, ot * PD:(ot + 1) * PD],
                        rhs[cc][:, n0:n0 + nsz],
                        start=(cc == 0), stop=(cc == NCC - 1))
                dst = outs[ot][:, n0:n0 + nsz]
                if bias is not None:
                    nc.scalar.activation(dst, p[:], AF.Identity, bias=bias[ot])
                else:
                    nc.vector.tensor_copy(dst, p[:])

    for b in range(BPC):
        # ---- load per-batch inputs ----
        xbt = xbts[b]
        if b > 0:
            for j in range(NCC):
                nc.sync.dma_start(xbt[:, j * HW:(j + 1) * HW],
                                  d["x"][b, j * PD:(j + 1) * PD, :])
        xb = [xbt[:, j * HW:(j + 1) * HW] for j in range(NCC)]
        tvt = act.tile([PD, NCC], F32, name=f"tv{b}", tag="tv")
        nc.sync.dma_start(tvt[:],
                          d["tvec"][b].rearrange("(cc p) one -> p (cc one)", p=PD))
        tvecs = [tvt[:, j:j + 1] for j in range(NCC)]
        tmbt = act.tile([PD, NCC * TMP], F32R, name=f"tmblk{b}", tag="tmblk")
        nc.sync.dma_start(tmbt[:].rearrange("p (cc h) -> p cc h", cc=NCC),
                          d["t_m_blk"][b].rearrange("(cc p) h -> p cc h", p=PD))
        tmblk = [tmbt[:, j * TMP:(j + 1) * TMP] for j in range(NCC)]

        # ---- Q, K, vl convs ----
        Q = [act.tile([PD, HW], F32R, name=f"q{b}_{j}", tag=f"q{j}", bufs=2)
             for j in range(NCC)]
        K = [act.tile([PD, HW], F32R, name=f"k{b}_{j}", tag=f"k{j}", bufs=2)
             for j in range(NCC)]
        vl = [act.tile([PD, HW], F32R, name=f"vl{b}_{j}", tag=f"vl{j}")
              for j in range(NCC)]
        conv("q", b, W["WqT"], xb, Q)
        conv("k", b, W["WkT"], xb, K)
        conv("vl", b, W["Wm1T"], xb, vl, bias=tvecs)

        # ---- V'T: (Wv @ vl)^T into the persistent padded tiles ----
        for mi, (m0, sz) in enumerate(MT):
            p = ps.tile([sz, C], F32, tag="conv", bufs=2, name=f"p_vt{b}_{mi}")
            for cc in range(NCC):
                nc.tensor.matmul(p[:], vl[cc][:, m0:m0 + sz], W["WvT"][cc][:],
                                 start=(cc == 0), stop=(cc == NCC - 1))
            vsrc = p[:].rearrange("p (h c) -> p h c", h=NH)
            vv = VT[mi][:].rearrange("p (h c) -> p h c", h=NH)
            nc.vector.tensor_copy(vv[:, :, 0:CPH], vsrc)

        # ---- cross attention softmax (over hw, free dim) ----
        crosse = act.tile([NH, HW], F32, name=f"crosse{b}", tag="crosse")
        csum = [act.tile([NH, 1], F32, name=f"csum{b}_{i}", tag=f"csum{i}")
                for i in range(2)]
        for hi, (n0, nsz) in enumerate(NHALF):
            p = ps.tile([TMP, nsz], F32, tag="s", bufs=3, name=f"p_cl{b}_{hi}")
            for cc in range(NCC):
                nc.tensor.matmul(p[:], tmblk[cc], xb[cc][:, n0:n0 + nsz],
                                 start=(cc == 0), stop=(cc == NCC - 1))
            nc.scalar.activation(crosse[:, n0:n0 + nsz], p[0:NH, :], AF.Exp,
                                 scale=SCALE, accum_out=csum[hi][:])
        crec = act.tile([NH, 1], F32, name=f"crec{b}", tag="crec")
        nc.vector.tensor_add(crec[:], csum[0][:], csum[1][:])
        nc.vector.reciprocal(crec[:], crec[:])
        crossn = act.tile([NH, HW], F32, name=f"crossn{b}", tag="crossn")
        nc.vector.tensor_scalar_mul(crossn[:], crosse[:], crec[:])
        # transpose to [hw, 9]; identity sliced [8,9] makes col 8 a zero pad
        # (the es fold needs a finite 578th column, value irrelevant).
        crossT = [act.tile([sz, NH + 1], F32R, name=f"crossT{b}_{mi}",
                           tag=f"crossT{mi}") for mi, (m0, sz) in enumerate(MT)]
        for mi, (m0, sz) in enumerate(MT):
            pt = ps.tile([sz, NH], F32, tag="conv", bufs=2,
                         name=f"p_ct{b}_{mi}")
            nc.tensor.transpose(pt[:], crossn[:, m0:m0 + sz], ident[0:NH, 0:NH])
            nc.vector.tensor_copy(crossT[mi][0:sz, 0:NH], pt[:])
            nc.gpsimd.tensor_copy(crossT[mi][0:sz, NH:NH + 1], onesb[0:sz, 0:1])

        # ---- per-head attention, heads processed in row-group pairs ----
        outall = [act.tile([PD, HW], F32R, name=f"oa{b}_{j}", tag=f"oa{j}")
                  for j in range(NCC)]
        for hp in range(NH // 2):
            h2 = (2 * hp, 2 * hp + 1)
            # es[sub][mi]: [sz, 578]; cols 0:576 = exp(scale*S^T), col 576 =
            # crossnorm (fused cross-value column), col 577 = finite pad.
            es = [[expp.tile([sz, ESW], F32R, name=f"es{b}_{hp}_{sub}_{mi}",
                             tag=f"es{sub}_{mi}", bufs=2)
                   for mi, (m0, sz) in enumerate(MT)] for sub in range(2)]
            # S^T + exp; even/odd head matmuls adjacent -> disjoint row groups
            # (rows 0:64 vs 64:128) run concurrently in the PE array.
            for mi, (m0, sz) in enumerate(MT):
                for hi, (n0, nsz) in enumerate(NHALF):
                    for sub in range(2):
                        rr = sub * CPH
                        p = ps.tile([sz, nsz], F32, tag="s", bufs=3,
                                    name=f"p_s{b}_{hp}_{sub}_{mi}_{n0}")
                        nc.tensor.matmul(
                            p[:], K[hp][rr:rr + CPH, m0:m0 + sz],
                            Q[hp][rr:rr + CPH, n0:n0 + nsz],
                            start=True, stop=True)
                        nc.scalar.activation(es[sub][mi][:, n0:n0 + nsz], p[:],
                                             AF.Exp, scale=SCALE)
                for sub in range(2):
                    nc.gpsimd.tensor_copy(
                        es[sub][mi][:, HW:ESW],
                        crossT[mi][0:sz, h2[sub]:h2[sub] + 2])
            # AV: out rows 0:64 = V_h @ A-parts, row 64 = softmax sums;
            # av1 col 64 (global 576) = cross-value vector.
            for sub in range(2):
                h = h2[sub]
                pav = [ps.tile([PD, nsz], F32, tag="av", bufs=3,
                               name=f"p_av{b}_{h}_{ci}")
                       for ci, (c0, nsz) in enumerate(AVCH)]
                for mi, (m0, sz) in enumerate(MT):
                    lhs = VT[mi][:, h * CPS:(h + 1) * CPS]
                    st, sp = (mi == 0), (mi == len(MT) - 1)
                    for ci, (c0, nsz) in enumerate(AVCH):
                        nc.tensor.matmul(pav[ci][:], lhs,
                                         es[sub][mi][:, c0:c0 + nsz],
                                         start=st, stop=sp)
                # normalize + add cross-value; write into outall rows rr:rr+64
                rr = sub * CPH
                vcr = act.tile([CPH, 1], F32, name=f"vcr{b}_{h}", tag="vcr",
                               bufs=2)
                nc.vector.tensor_copy(vcr[:], pav[1][0:CPH, 288:289])
                rcrow = act.tile([1, HW], F32, name=f"rc{b}_{h}", tag="rcrow",
                                 bufs=2)
                rcln = act.tile([1, HW], F32, name=f"rl{b}_{h}", tag="rcln",
                                bufs=1)
                rep = act.tile([CPH, HW], F32, name=f"rep{b}_{h}", tag="rep",
                               bufs=2)
                # 1/sums via exp(-ln(sums)) on ACT: ~5x cheaper than the
                # lane-serial DVE reciprocal on a [1, N] row.
                nc.scalar.activation(rcln[:, 0:288], pav[0][CPH:CPH + 1, :],
                                     AF.Ln)
                nc.scalar.activation(rcln[:, 288:HW],
                                     pav[1][CPH:CPH + 1, 0:288], AF.Ln)
                nc.scalar.activation(rcrow[:], rcln[:], AF.Exp, scale=-1.0)
                nc.gpsimd.partition_broadcast(rep[:], rcrow[:])
                dst = outall[hp][rr:rr + CPH, :]
                nc.vector.tensor_tensor(dst[:, 0:288], pav[0][0:CPH, :],
                                        rep[:, 0:288], OP.mult)
                nc.vector.tensor_tensor(dst[:, 288:HW], pav[1][0:CPH, 0:288],
                                        rep[:, 288:HW], OP.mult)
                nc.vector.tensor_scalar_add(dst, dst, vcr[:])

        # ---- final conv + bias, DMA out ----
        fin = [act.tile([PD, HW], F32, name=f"fin{b}_{j}", tag=f"fin{j}")
               for j in range(NCC)]
        conv("fin", b, W["WrT"], outall, fin, bias=wrb)
        for ot in range(NCC):
            nc.sync.dma_start(d["out"][b, ot * PD:(ot + 1) * PD, :], fin[ot][:])


_CACHE = {}


def _build():
    if "nc" in _CACHE:
        return _CACHE["nc"], _CACHE["out"]
    nc = bacc.Bacc("TRN2", target_bir_lowering=False, debug=False,
                   num_devices=NCORES)
    d = {
        "x": nc.dram_tensor("x", [BPC, C, HW], F32R, kind="ExternalInput").ap(),
        "t_m_blk": nc.dram_tensor("t_m_blk", [BPC, C, TMP], F32R,
                                  kind="ExternalInput").ap(),
        "tvec": nc.dram_tensor("tvec", [BPC, C, 1], F32, kind="ExternalInput").ap(),
        "Wr_b": nc.dram_tensor("Wr_b", [C, 1], F32, kind="ExternalInput").ap(),
        "out": nc.dram_tensor("out", [BPC, C, HW], F32, kind="ExternalOutput").ap(),
    }
    for wn in ("WqT", "WkT", "WvT", "Wm1T", "WrT"):
        d[wn] = nc.dram_tensor(wn, [C, C], F32R, kind="ExternalInput").ap()
    with tile.TileContext(nc) as tc:
        with ExitStack() as ctx:
            _body(ctx, tc, d)
    nc.compile()
    _CACHE["nc"], _CACHE["out"] = nc, d["out"].tensor.name
    return nc, _CACHE["out"]


def _prep_inputs(x, t, Wk, Wq, Wt_w, Wt_b, Wm, Wv, Wr_w, Wr_b):
    f = np.float32
    x = np.asarray(x, f).reshape(B, C, HW)
    t = np.asarray(t, f)
    t_m = t @ np.asarray(Wt_w, f).T + np.asarray(Wt_b, f)
    t_m_blk = np.zeros((B, C, TMP), f)
    for h in range(NH):
        t_m_blk[:, h * CPH:(h + 1) * CPH, h] = t_m[:, h * CPH:(h + 1) * CPH]
    tvec = (t @ np.asarray(Wm, f)[:, C:].T).reshape(B, C, 1)
    com = {
        "WqT": np.ascontiguousarray(np.asarray(Wq, f).T),
        "WkT": np.ascontiguousarray(np.asarray(Wk, f).T),
        "WvT": np.ascontiguousarray(np.asarray(Wv, f).T),
        "Wm1T": np.ascontiguousarray(np.asarray(Wm, f)[:, :C].T),
        "WrT": np.ascontiguousarray(np.asarray(Wr_w, f).T),
        "Wr_b": np.asarray(Wr_b, f).reshape(C, 1),
    }
    maps = []
    for c in range(NCORES):
        sl = slice(c * BPC, (c + 1) * BPC)
        m = dict(com)
        m["x"] = np.ascontiguousarray(x[sl])
        m["t_m_blk"] = np.ascontiguousarray(t_m_blk[sl])
        m["tvec"] = np.ascontiguousarray(tvec[sl])
        maps.append(m)
    return maps


def kernel(x, t, Wk, Wq, Wt_w, Wt_b, Wm, Wv, Wr_w, Wr_b, _trace=False):
    nc, out_name = _build()
    maps = _prep_inputs(x, t, Wk, Wq, Wt_w, Wt_b, Wm, Wv, Wr_w, Wr_b)
    res = run_bass_kernel_spmd(nc, maps, core_ids=list(range(NCORES)),
                               trace=_trace)
    out = np.concatenate([res.results[c][out_name] for c in range(NCORES)],
                         axis=0).reshape(B, C, 24, 24)
    if _trace:
        kernel.last_results = res
    return out


# revision 17
# speedup vs baseline: 1.0645x; 1.0645x over previous
"""Trainium2 Bass kernel for nn_AttentionModule_50002009260608.

B=16, C=512, H=W=24 (HW=576), TF=512, NH=8, CPH=64.
Data-parallel over batch: 2 batch elements per core x 8 cores.
Weights replicated; host pre-transposes 1x1-conv weights to [c_in, c_out]
and precomputes the two tiny text matvecs (t_m, Wm2 @ t).

All heavy matmuls run as float32r (full PE rate for N>=256) accumulating
in fp32 PSUM. fp32r ISA restrictions: output must span all 128 PE columns
(M>=97) and innermost AP counts must be even -- hence hw m-tiles of
116+115*4 and the padded per-head V'T stride of 128.
"""

import numpy as np
from contextlib import ExitStack

import concourse.bacc as bacc
import concourse.bass as bass
import concourse.tile as tile
import concourse.mybir as mybir
from concourse import masks
from concourse.bass_utils import run_bass_kernel_spmd

B, C, HW, TF, NH, CPH = 16, 512, 576, 512, 8, 64
NCORES, BPC = 8, B // 8
SCALE = 1.0 / 8.0  # 1/sqrt(CPH)
F32, F32R = mybir.dt.float32, mybir.dt.float32r
AF = mybir.ActivationFunctionType
OP = mybir.AluOpType
PD = 128
NCC = C // PD                                    # 4 channel chunks
MT = [(0, 116), (116, 115), (231, 115), (346, 115), (461, 115)]  # hw m-tiles
NHALF = [(0, 288), (288, 288)]                   # softmax eviction halves
AVCH = [(0, 288), (288, 290)]                    # AV rhs chunks over es cols
CPS = 128                                        # padded per-head V'T stride
TMP = 104                                        # padded t_m_blk cols (fp32r M>=97)
ESW = HW + 2                                     # es cols: 576 + cross col + pad


def _body(ctx: ExitStack, tc, d):
    """d: DRAM APs: x[2,512,576](f32r), t_m_blk[2,512,104](f32r),
    tvec[2,512,1], WqT/WkT/WvT/Wm1T/WrT [512,512](f32r, pre-transposed
    [c_in,c_out]), Wr_b[512,1], out[2,512,576]."""
    nc = tc.nc

    wt = ctx.enter_context(tc.tile_pool(name="wt", bufs=1))
    act = ctx.enter_context(tc.tile_pool(name="act", bufs=1))
    expp = ctx.enter_context(tc.tile_pool(name="expp", bufs=1))
    ps = ctx.enter_context(tc.tile_pool(name="ps", bufs=1, space="PSUM"))

    # ---- batch-0 activations first (PE can start within ~2us), then weights,
    # all split per channel-chunk so the first conv group's deps arrive early ----
    xbts = []
    for b in range(BPC):
        xbt = act.tile([PD, NCC * HW], F32R, name=f"xb{b}", tag="xb", bufs=2)
        if b == 0:
            for j in range(NCC):
                nc.sync.dma_start(xbt[:, j * HW:(j + 1) * HW],
                                  d["x"][b, j * PD:(j + 1) * PD, :])
        xbts.append(xbt)
    W = {}
    for wn in ("WqT", "WkT", "Wm1T", "WvT", "WrT"):
        wtile = wt.tile([PD, NCC * C], F32R, name=f"{wn}_t")
        for j in range(NCC):
            nc.sync.dma_start(wtile[:, j * C:(j + 1) * C],
                              d[wn][j * PD:(j + 1) * PD, :])
        W[wn] = [wtile[:, j * C:(j + 1) * C] for j in range(NCC)]
    wrbt = wt.tile([PD, NCC], F32, name="wrbt")
    nc.sync.dma_start(wrbt[:], d["Wr_b"].rearrange("(cc p) one -> p (cc one)", p=PD))
    wrb = [wrbt[:, j:j + 1] for j in range(NCC)]
    ident = wt.tile([PD, PD], F32, name="ident")
    masks.make_identity(nc, ident[:])
    onesb = wt.tile([PD, (CPS - CPH) * NH], F32, name="onesb")
    nc.vector.memset(onesb[:], 1.0)
    # persistent V'T tiles: [hw_tile, 8*128]; per head block: cols 0:64 = V_h^T,
    # cols 64:128 = 1.0 (fused softmax column sums). Ones written once.
    VT = [wt.tile([sz, NH * CPS], F32R, name=f"vt{mi}")
          for mi, (m0, sz) in enumerate(MT)]
    for mi, (m0, sz) in enumerate(MT):
        nc.vector.tensor_copy(
            VT[mi][:].rearrange("p (h c) -> p h c", h=NH)[:, :, CPH:CPS],
            onesb[0:sz, :])

    def conv(name, b, Wn, rhs, outs, bias=None):
        # outs[ot][:, n] = sum_cc Wn[cc][:, ot*128:+128].T @ rhs[cc][:, n] (+ bias)
        for ot in range(NCC):
            for (n0, nsz) in NHALF:
                p = ps.tile([PD, nsz], F32, tag="conv", bufs=2,
                            name=f"p_{name}{b}_{ot}_{n0}")
                for cc in range(NCC):
                    nc.tensor.matmul(
                        p[:], Wn[cc][:, ot * PD:(ot + 1) * PD],
                        rhs[cc][:, n0:n0 + nsz],
                        start=(cc == 0), stop=(cc == NCC - 1))
                dst = outs[ot][:, n0:n0 + nsz]
                if bias is not None:
                    nc.scalar.activation(dst, p[:], AF.Identity, bias=bias[ot])
                else:
                    nc.vector.tensor_copy(dst, p[:])

    for b in range(BPC):
        # ---- load per-batch inputs ----
        xbt = xbts[b]
        if b > 0:
            for j in range(NCC):
                nc.sync.dma_start(xbt[:, j * HW:(j + 1) * HW],
                                  d["x"][b, j * PD:(j + 1) * PD, :])
        xb = [xbt[:, j * HW:(j + 1) * HW] for j in range(NCC)]
        tvt = act.tile([PD, NCC], F32, name=f"tv{b}", tag="tv")
        nc.sync.dma_start(tvt[:],
                          d["tvec"][b].rearrange("(cc p) one -> p (cc one)", p=PD))
        tvecs = [tvt[:, j:j + 1] for j in range(NCC)]
        tmbt = act.tile([PD, NCC * TMP], F32R, name=f"tmblk{b}", tag="tmblk")
        nc.sync.dma_start(tmbt[:].rearrange("p (cc h) -> p cc h", cc=NCC),
                          d["t_m_blk"][b].rearrange("(cc p) h -> p cc h", p=PD))
        tmblk = [tmbt[:, j * TMP:(j + 1) * TMP] for j in range(NCC)]

        # ---- Q, K, vl convs ----
        Q = [act.tile([PD, HW], F32R, name=f"q{b}_{j}", tag=f"q{j}", bufs=2)
             for j in range(NCC)]
        K = [act.tile([PD, HW], F32R, name=f"k{b}_{j}", tag=f"k{j}", bufs=2)
             for j in range(NCC)]
        vl = [act.tile([PD, HW], F32R, name=f"vl{b}_{j}", tag=f"vl{j}")
              for j in range(NCC)]
        conv("q", b, W["WqT"], xb, Q)
        conv("k", b, W["WkT"], xb, K)
        conv("vl", b, W["Wm1T"], xb, vl, bias=tvecs)

        # ---- V'T: (Wv @ vl)^T into the persistent padded tiles ----
        for mi, (m0, sz) in enumerate(MT):
            p = ps.tile([sz, C], F32, tag="conv", bufs=2, name=f"p_vt{b}_{mi}")
            for cc in range(NCC):
                nc.tensor.matmul(p[:], vl[cc][:, m0:m0 + sz], W["WvT"][cc][:],
                                 start=(cc == 0), stop=(cc == NCC - 1))
            vsrc = p[:].rearrange("p (h c) -> p h c", h=NH)
            vv = VT[mi][:].rearrange("p (h c) -> p h c", h=NH)
            nc.vector.tensor_copy(vv[:, :, 0:CPH], vsrc)

        # ---- cross attention softmax (over hw, free dim) ----
        crosse = act.tile([NH, HW], F32, name=f"crosse{b}", tag="crosse")
        csum = [act.tile([NH, 1], F32, name=f"csum{b}_{i}", tag=f"csum{i}")
                for i in range(2)]
        for hi, (n0, nsz) in enumerate(NHALF):
            p = ps.tile([TMP, nsz], F32, tag="s", bufs=3, name=f"p_cl{b}_{hi}")
            for cc in range(NCC):
                nc.tensor.matmul(p[:], tmblk[cc], xb[cc][:, n0:n0 + nsz],
                                 start=(cc == 0), stop=(cc == NCC - 1))
            nc.scalar.activation(crosse[:, n0:n0 + nsz], p[0:NH, :], AF.Exp,
                                 scale=SCALE, accum_out=csum[hi][:])
        crec = act.tile([NH, 1], F32, name=f"crec{b}", tag="crec")
        nc.vector.tensor_add(crec[:], csum[0][:], csum[1][:])
        nc.vector.reciprocal(crec[:], crec[:])
        crossn = act.tile([NH, HW], F32, name=f"crossn{b}", tag="crossn")
        nc.vector.tensor_scalar_mul(crossn[:], crosse[:], crec[:])
        # transpose to [hw, 9]; identity sliced [8,9] makes col 8 a zero pad
        # (the es fold needs a finite 578th column, value irrelevant).
        crossT = [act.tile([sz, NH + 1], F32R, name=f"crossT{b}_{mi}",
                           tag=f"crossT{mi}") for mi, (m0, sz) in enumerate(MT)]
        for mi, (m0, sz) in enumerate(MT):
            pt = ps.tile([sz, NH], F32, tag="conv", bufs=2,
                         name=f"p_ct{b}_{mi}")
            nc.tensor.transpose(pt[:], crossn[:, m0:m0 + sz], ident[0:NH, 0:NH])
            nc.vector.tensor_copy(crossT[mi][0:sz, 0:NH], pt[:])
            nc.gpsimd.tensor_copy(crossT[mi][0:sz, NH:NH + 1], onesb[0:sz, 0:1])

        # ---- per-head attention, heads processed in row-group pairs ----
        outall = [act.tile([PD, HW], F32R, name=f"oa{b}_{j}", tag=f"oa{j}")
                  for j in range(NCC)]
        for hp in range(NH // 2):
            h2 = (2 * hp, 2 * hp + 1)
            # es[sub][mi]: [sz, 578]; cols 0:576 = exp(scale*S^T), col 576 =
            # crossnorm (fused cross-value column), col 577 = finite pad.
            es = [[expp.tile([sz, ESW], F32R, name=f"es{b}_{hp}_{sub}_{mi}",
                             tag=f"es{sub}_{mi}", bufs=2)
                   for mi, (m0, sz) in enumerate(MT)] for sub in range(2)]
            # S^T + exp; even/odd head matmuls adjacent -> disjoint row groups
            # (rows 0:64 vs 64:128) run concurrently in the PE array.
            for mi, (m0, sz) in enumerate(MT):
                for hi, (n0, nsz) in enumerate(NHALF):
                    for sub in range(2):
                        rr = sub * CPH
                        p = ps.tile([sz, nsz], F32, tag="s", bufs=3,
                                    name=f"p_s{b}_{hp}_{sub}_{mi}_{n0}")
                        nc.tensor.matmul(
                            p[:], K[hp][rr:rr + CPH, m0:m0 + sz],
                            Q[hp][rr:rr + CPH, n0:n0 + nsz],
                            start=True, stop=True)
                        nc.scalar.activation(es[sub][mi][:, n0:n0 + nsz], p[:],
                                             AF.Exp, scale=SCALE)
                for sub in range(2):
                    nc.gpsimd.tensor_copy(
                        es[sub][mi][:, HW:ESW],
                        crossT[mi][0:sz, h2[sub]:h2[sub] + 2])
            # AV: out rows 0:64 = V_h @ A-parts, row 64 = softmax sums;
            # av1 col 64 (global 576) = cross-value vector.
            for sub in range(2):
                h = h2[sub]
                pav = [ps.tile([PD, nsz], F32, tag="av", bufs=3,
                               name=f"p_av{b}_{h}_{ci}")
                       for ci, (c0, nsz) in enumerate(AVCH)]
                for mi, (m0, sz) in enumerate(MT):
                    lhs = VT[mi][:, h * CPS:(h + 1) * CPS]
                    st, sp = (mi == 0), (mi == len(MT) - 1)
                    for ci, (c0, nsz) in enumerate(AVCH):
                        nc.tensor.matmul(pav[ci][:], lhs,
                                         es[sub][mi][:, c0:c0 + nsz],
                                         start=st, stop=sp)
                # normalize + add cross-value; write into outall rows rr:rr+64
                rr = sub * CPH
                vcr = act.tile([CPH, 1], F32, name=f"vcr{b}_{h}", tag="vcr",
                               bufs=2)
                nc.vector.tensor_copy(vcr[:], pav[1][0:CPH, 288:289])
                rcrow = act.tile([1, HW], F32, name=f"rc{b}_{h}", tag="rcrow",
                                 bufs=2)
                rep = act.tile([CPH, HW], F32, name=f"rep{b}_{h}", tag="rep",
                               bufs=2)
                nc.vector.reciprocal(rcrow[:, 0:288], pav[0][CPH:CPH + 1, :])
                nc.vector.reciprocal(rcrow[:, 288:HW],
                                     pav[1][CPH:CPH + 1, 0:288])
                nc.gpsimd.partition_broadcast(rep[:], rcrow[:])
                dst = outall[hp][rr:rr + CPH, :]
                nc.vector.tensor_tensor(dst[:, 0:288], pav[0][0:CPH, :],
                                        rep[:, 0:288], OP.mult)
                nc.vector.tensor_tensor(dst[:, 288:HW], pav[1][0:CPH, 0:288],
                                        rep[:, 288:HW], OP.mult)
                nc.vector.tensor_scalar_add(dst, dst, vcr[:])

        # ---- final conv + bias, DMA out ----
        fin = [act.tile([PD, HW], F32, name=f"fin{b}_{j}", tag=f"fin{j}")
               for j in range(NCC)]
        conv("fin", b, W["WrT"], outall, fin, bias=wrb)
        for ot in range(NCC):
            nc.sync.dma_start(d["out"][b, ot * PD:(ot + 1) * PD, :], fin[ot][:])


_CACHE = {}


def _build():
    if "nc" in _CACHE:
        return _CACHE["nc"], _CACHE["out"]
    nc = bacc.Bacc("TRN2", target_bir_lowering=False, debug=False,
                   num_devices=NCORES)
    d = {
        "x": nc.dram_tensor("x", [BPC, C, HW], F32R, kind="ExternalInput").ap(),
        "t_m_blk": nc.dram_tensor("t_m_blk", [BPC, C, TMP], F32R,
                                  kind="ExternalInput").ap(),
        "tvec": nc.dram_tensor("tvec", [BPC, C, 1], F32, kind="ExternalInput").ap(),
        "Wr_b": nc.dram_tensor("Wr_b", [C, 1], F32, kind="ExternalInput").ap(),
        "out": nc.dram_tensor("out", [BPC, C, HW], F32, kind="ExternalOutput").ap(),
    }
    for wn in ("WqT", "WkT", "WvT", "Wm1T", "WrT"):
        d[wn] = nc.dram_tensor(wn, [C, C], F32R, kind="ExternalInput").ap()
    with tile.TileContext(nc) as tc:
        with ExitStack() as ctx:
            _body(ctx, tc, d)
    nc.compile()
    _CACHE["nc"], _CACHE["out"] = nc, d["out"].tensor.name
    return nc, _CACHE["out"]


def _prep_inputs(x, t, Wk, Wq, Wt_w, Wt_b, Wm, Wv, Wr_w, Wr_b):
    f = np.float32
    x = np.asarray(x, f).reshape(B, C, HW)
    t = np.asarray(t, f)
    t_m = t @ np.asarray(Wt_w, f).T + np.asarray(Wt_b, f)
    t_m_blk = np.zeros((B, C, TMP), f)
    for h in range(NH):
        t_m_blk[:, h * CPH:(h + 1) * CPH, h] = t_m[:, h * CPH:(h + 1) * CPH]
    tvec = (t @ np.asarray(Wm, f)[:, C:].T).reshape(B, C, 1)
    com = {
        "WqT": np.ascontiguousarray(np.asarray(Wq, f).T),
        "WkT": np.ascontiguousarray(np.asarray(Wk, f).T),
        "WvT": np.ascontiguousarray(np.asarray(Wv, f).T),
        "Wm1T": np.ascontiguousarray(np.asarray(Wm, f)[:, :C].T),
        "WrT": np.ascontiguousarray(np.asarray(Wr_w, f).T),
        "Wr_b": np.asarray(Wr_b, f).reshape(C, 1),
    }
    maps = []
    for c in range(NCORES):
        sl = slice(c * BPC, (c + 1) * BPC)
        m = dict(com)
        m["x"] = np.ascontiguousarray(x[sl])
        m["t_m_blk"] = np.ascontiguousarray(t_m_blk[sl])
        m["tvec"] = np.ascontiguousarray(tvec[sl])
        maps.append(m)
    return maps


def kernel(x, t, Wk, Wq, Wt_w, Wt_b, Wm, Wv, Wr_w, Wr_b, _trace=False):
    nc, out_name = _build()
    maps = _prep_inputs(x, t, Wk, Wq, Wt_w, Wt_b, Wm, Wv, Wr_w, Wr_b)
    res = run_bass_kernel_spmd(nc, maps, core_ids=list(range(NCORES)),
                               trace=_trace)
    out = np.concatenate([res.results[c][out_name] for c in range(NCORES)],
                         axis=0).reshape(B, C, 24, 24)
    if _trace:
        kernel.last_results = res
    return out


# revision 24
# speedup vs baseline: 1.1524x; 1.0826x over previous
"""Trainium2 Bass kernel for nn_AttentionModule_50002009260608.

B=16, C=512, H=W=24 (HW=576), TF=512, NH=8, CPH=64.
Data-parallel over batch: 2 batch elements per core x 8 cores.
Weights replicated; host pre-transposes 1x1-conv weights to [c_in, c_out]
and precomputes the two tiny text matvecs (t_m, Wm2 @ t).

All heavy matmuls run as float32r (full PE rate for N>=256) accumulating
in fp32 PSUM. fp32r ISA restrictions: output must span all 128 PE columns
(M>=97) and innermost AP counts must be even -- hence hw m-tiles of
116+115*4 and the padded per-head V'T stride of 128.
"""

import numpy as np
from contextlib import ExitStack

import concourse.bacc as bacc
import concourse.bass as bass
import concourse.tile as tile
import concourse.mybir as mybir
from concourse import masks
from concourse.bass_utils import run_bass_kernel_spmd

B, C, HW, TF, NH, CPH = 16, 512, 576, 512, 8, 64
NCORES, BPC = 8, B // 8
SCALE = 1.0 / 8.0  # 1/sqrt(CPH)
F32, F32R = mybir.dt.float32, mybir.dt.float32r
AF = mybir.ActivationFunctionType
OP = mybir.AluOpType
PD = 128
NCC = C // PD                                    # 4 channel chunks
MT = [(0, 116), (116, 115), (231, 115), (346, 115), (461, 115)]  # hw m-tiles
NHALF = [(0, 288), (288, 288)]                   # softmax eviction halves
AVCH = [(0, 288), (288, 290)]                    # AV rhs chunks over es cols
CPS = 128                                        # padded per-head V'T stride
TMP = 104                                        # padded t_m_blk cols (fp32r M>=97)
ESW = HW + 2                                     # es cols: 576 + cross col + pad


def _body(ctx: ExitStack, tc, d):
    """d: DRAM APs: x[2,512,576](f32r), t_m_blk[2,512,104](f32r),
    tvec[2,512,1], WqT/WkT/WvT/Wm1T/WrT [512,512](f32r, pre-transposed
    [c_in,c_out]), Wr_b[512,1], out[2,512,576]."""
    nc = tc.nc

    wt = ctx.enter_context(tc.tile_pool(name="wt", bufs=1))
    act = ctx.enter_context(tc.tile_pool(name="act", bufs=1))
    expp = ctx.enter_context(tc.tile_pool(name="expp", bufs=1))
    ps = ctx.enter_context(tc.tile_pool(name="ps", bufs=1, space="PSUM"))

    # ---- batch-0 activations first (PE can start within ~2us), then weights,
    # all split per channel-chunk so the first conv group's deps arrive early ----
    xbts = []
    for b in range(BPC):
        xbt = act.tile([PD, NCC * HW], F32R, name=f"xb{b}", tag="xb", bufs=2)
        if b == 0:
            for j in range(NCC):
                nc.sync.dma_start(xbt[:, j * HW:(j + 1) * HW],
                                  d["x"][b, j * PD:(j + 1) * PD, :])
        xbts.append(xbt)
    W = {}
    for wn in ("WqT", "WkT", "Wm1T", "WvT", "WrT"):
        wtile = wt.tile([PD, NCC * C], F32R, name=f"{wn}_t")
        for j in range(NCC):
            nc.sync.dma_start(wtile[:, j * C:(j + 1) * C],
                              d[wn][j * PD:(j + 1) * PD, :])
        W[wn] = [wtile[:, j * C:(j + 1) * C] for j in range(NCC)]
    wrbt = wt.tile([PD, NCC], F32, name="wrbt")
    nc.sync.dma_start(wrbt[:], d["Wr_b"].rearrange("(cc p) one -> p (cc one)", p=PD))
    wrb = [wrbt[:, j:j + 1] for j in range(NCC)]
    ident = wt.tile([PD, PD], F32, name="ident")
    masks.make_identity(nc, ident[:])
    onesb = wt.tile([PD, (CPS - CPH) * NH], F32, name="onesb")
    nc.vector.memset(onesb[:], 1.0)
    # batched softmax-sum reciprocal scratch: a sub-head's 2 sums rows parked
    # at partitions 0/32, one [33, 288] reciprocal covers both halves.
    # Two tiles, alternated by sub parity, so consecutive sub-heads pipeline.
    smt2 = [wt.tile([33, 288], F32, name=f"smt{i}") for i in range(2)]
    smr2 = [wt.tile([33, 288], F32, name=f"smr{i}") for i in range(2)]
    smb2 = [wt.tile([1, 288], F32, name=f"smb{i}") for i in range(2)]
    for i in range(2):
        nc.vector.memset(smt2[i][:], 1.0)
    # persistent V'T tiles: [hw_tile, 8*128]; per head block: cols 0:64 = V_h^T,
    # cols 64:128 = 1.0 (fused softmax column sums). Ones written once.
    VT = [wt.tile([sz, NH * CPS], F32R, name=f"vt{mi}")
          for mi, (m0, sz) in enumerate(MT)]
    for mi, (m0, sz) in enumerate(MT):
        nc.vector.tensor_copy(
            VT[mi][:].rearrange("p (h c) -> p h c", h=NH)[:, :, CPH:CPS],
            onesb[0:sz, :])

    def conv(name, b, Wn, rhs, outs, bias=None):
        # outs[ot][:, n] = sum_cc Wn[cc][:, ot*128:+128].T @ rhs[cc][:, n] (+ bias)
        for ot in range(NCC):
            for (n0, nsz) in NHALF:
                p = ps.tile([PD, nsz], F32, tag="conv", bufs=2,
                            name=f"p_{name}{b}_{ot}_{n0}")
                for cc in range(NCC):
                    nc.tensor.matmul(
                        p[:], Wn[cc][:, ot * PD:(ot + 1) * PD],
                        rhs[cc][:, n0:n0 + nsz],
                        start=(cc == 0), stop=(cc == NCC - 1))
                dst = outs[ot][:, n0:n0 + nsz]
                if bias is not None:
                    nc.scalar.activation(dst, p[:], AF.Identity, bias=bias[ot])
                else:
                    nc.vector.tensor_copy(dst, p[:])

    for b in range(BPC):
        # ---- load per-batch inputs ----
        xbt = xbts[b]
        if b > 0:
            for j in range(NCC):
                nc.sync.dma_start(xbt[:, j * HW:(j + 1) * HW],
                                  d["x"][b, j * PD:(j + 1) * PD, :])
        xb = [xbt[:, j * HW:(j + 1) * HW] for j in range(NCC)]
        tvt = act.tile([PD, NCC], F32, name=f"tv{b}", tag="tv")
        nc.sync.dma_start(tvt[:],
                          d["tvec"][b].rearrange("(cc p) one -> p (cc one)", p=PD))
        tvecs = [tvt[:, j:j + 1] for j in range(NCC)]
        tmbt = act.tile([PD, NCC * TMP], F32R, name=f"tmblk{b}", tag="tmblk")
        nc.sync.dma_start(tmbt[:].rearrange("p (cc h) -> p cc h", cc=NCC),
                          d["t_m_blk"][b].rearrange("(cc p) h -> p cc h", p=PD))
        tmblk = [tmbt[:, j * TMP:(j + 1) * TMP] for j in range(NCC)]

        # ---- Q, K, vl convs ----
        Q = [act.tile([PD, HW], F32R, name=f"q{b}_{j}", tag=f"q{j}", bufs=2)
             for j in range(NCC)]
        K = [act.tile([PD, HW], F32R, name=f"k{b}_{j}", tag=f"k{j}", bufs=2)
             for j in range(NCC)]
        vl = [act.tile([PD, HW], F32R, name=f"vl{b}_{j}", tag=f"vl{j}")
              for j in range(NCC)]
        conv("q", b, W["WqT"], xb, Q)
        conv("k", b, W["WkT"], xb, K)
        conv("vl", b, W["Wm1T"], xb, vl, bias=tvecs)

        # ---- V'T: (Wv @ vl)^T into the persistent padded tiles ----
        for mi, (m0, sz) in enumerate(MT):
            p = ps.tile([sz, C], F32, tag="conv", bufs=2, name=f"p_vt{b}_{mi}")
            for cc in range(NCC):
                nc.tensor.matmul(p[:], vl[cc][:, m0:m0 + sz], W["WvT"][cc][:],
                                 start=(cc == 0), stop=(cc == NCC - 1))
            vsrc = p[:].rearrange("p (h c) -> p h c", h=NH)
            vv = VT[mi][:].rearrange("p (h c) -> p h c", h=NH)
            nc.vector.tensor_copy(vv[:, :, 0:CPH], vsrc)

        # ---- cross attention softmax (over hw, free dim) ----
        crosse = act.tile([NH, HW], F32, name=f"crosse{b}", tag="crosse")
        csum = [act.tile([NH, 1], F32, name=f"csum{b}_{i}", tag=f"csum{i}")
                for i in range(2)]
        for hi, (n0, nsz) in enumerate(NHALF):
            p = ps.tile([TMP, nsz], F32, tag="s", bufs=3, name=f"p_cl{b}_{hi}")
            for cc in range(NCC):
                nc.tensor.matmul(p[:], tmblk[cc], xb[cc][:, n0:n0 + nsz],
                                 start=(cc == 0), stop=(cc == NCC - 1))
            nc.scalar.activation(crosse[:, n0:n0 + nsz], p[0:NH, :], AF.Exp,
                                 scale=SCALE, accum_out=csum[hi][:])
        crec = act.tile([NH, 1], F32, name=f"crec{b}", tag="crec")
        nc.vector.tensor_add(crec[:], csum[0][:], csum[1][:])
        nc.vector.reciprocal(crec[:], crec[:])
        crossn = act.tile([NH, HW], F32, name=f"crossn{b}", tag="crossn")
        nc.vector.tensor_scalar_mul(crossn[:], crosse[:], crec[:])
        # transpose to [hw, 9]; identity sliced [8,9] makes col 8 a zero pad
        # (the es fold needs a finite 578th column, value irrelevant).
        crossT = [act.tile([sz, NH + 1], F32R, name=f"crossT{b}_{mi}",
                           tag=f"crossT{mi}") for mi, (m0, sz) in enumerate(MT)]
        for mi, (m0, sz) in enumerate(MT):
            pt = ps.tile([sz, NH], F32, tag="conv", bufs=2,
                         name=f"p_ct{b}_{mi}")
            nc.tensor.transpose(pt[:], crossn[:, m0:m0 + sz], ident[0:NH, 0:NH])
            nc.vector.tensor_copy(crossT[mi][0:sz, 0:NH], pt[:])
            nc.gpsimd.tensor_copy(crossT[mi][0:sz, NH:NH + 1], onesb[0:sz, 0:1])

        # ---- per-head attention, heads processed in row-group pairs ----
        outall = [act.tile([PD, HW], F32R, name=f"oa{b}_{j}", tag=f"oa{j}")
                  for j in range(NCC)]
        for hp in range(NH // 2):
            h2 = (2 * hp, 2 * hp + 1)
            # es[sub][mi]: [sz, 578]; cols 0:576 = exp(scale*S^T), col 576 =
            # crossnorm (fused cross-value column), col 577 = finite pad.
            es = [[expp.tile([sz, ESW], F32R, name=f"es{b}_{hp}_{sub}_{mi}",
                             tag=f"es{sub}_{mi}", bufs=2)
                   for mi, (m0, sz) in enumerate(MT)] for sub in range(2)]
            # S^T + exp; even/odd head matmuls adjacent -> disjoint row groups
            # (rows 0:64 vs 64:128) run concurrently in the PE array.
            for mi, (m0, sz) in enumerate(MT):
                for hi, (n0, nsz) in enumerate(NHALF):
                    for sub in range(2):
                        rr = sub * CPH
                        p = ps.tile([sz, nsz], F32, tag="s", bufs=3,
                                    name=f"p_s{b}_{hp}_{sub}_{mi}_{n0}")
                        nc.tensor.matmul(
                            p[:], K[hp][rr:rr + CPH, m0:m0 + sz],
                            Q[hp][rr:rr + CPH, n0:n0 + nsz],
                            start=True, stop=True)
                        nc.scalar.activation(es[sub][mi][:, n0:n0 + nsz], p[:],
                                             AF.Exp, scale=SCALE)
                for sub in range(2):
                    nc.gpsimd.tensor_copy(
                        es[sub][mi][:, HW:ESW],
                        crossT[mi][0:sz, h2[sub]:h2[sub] + 2])
            # AV: out rows 0:64 = V_h @ A-parts, row 64 = softmax sums;
            # av1 col 288 (global es col 576) = cross-value vector.
            for sub in range(2):
                h = h2[sub]
                pav = [ps.tile([PD, nsz], F32, tag="av", bufs=3,
                               name=f"p_av{b}_{h}_{ci}")
                       for ci, (c0, nsz) in enumerate(AVCH)]
                for mi, (m0, sz) in enumerate(MT):
                    lhs = VT[mi][:, h * CPS:(h + 1) * CPS]
                    st, sp = (mi == 0), (mi == len(MT) - 1)
                    for ci, (c0, nsz) in enumerate(AVCH):
                        nc.tensor.matmul(pav[ci][:], lhs,
                                         es[sub][mi][:, c0:c0 + nsz],
                                         start=st, stop=sp)
                # batched reciprocal of both halves' sums (rows 0 and 32)
                rr = sub * CPH
                smt = smt2[(2 * hp + sub) % 2]
                smr = smr2[(2 * hp + sub) % 2]
                smb = smb2[(2 * hp + sub) % 2]
                nc.vector.tensor_copy(smt[0:1, :], pav[0][CPH:CPH + 1, 0:288])
                nc.vector.tensor_copy(smt[32:33, :], pav[1][CPH:CPH + 1, 0:288])
                nc.vector.reciprocal(smr[:], smt[:])
                # partition_broadcast reads physical partition 0 only: move the
                # half-1 reciprocal row from partition 32 back to base 0.
                nc.vector.tensor_copy(smb[:], smr[32:33, :])
                vcr = act.tile([CPH, 1], F32, name=f"vcr{b}_{h}", tag="vcr",
                               bufs=2)
                nc.vector.tensor_copy(vcr[:], pav[1][0:CPH, 288:289])
                rep = act.tile([CPH, HW], F32, name=f"rep{b}_{h}", tag="rep",
                               bufs=2)
                nc.gpsimd.partition_broadcast(rep[:, 0:288], smr[0:1, :])
                nc.gpsimd.partition_broadcast(rep[:, 288:HW], smb[:])
                dst = outall[hp][rr:rr + CPH, :]
                nc.vector.tensor_tensor(dst[:, 0:288], pav[0][0:CPH, :],
                                        rep[:, 0:288], OP.mult)
                nc.vector.tensor_tensor(dst[:, 288:HW], pav[1][0:CPH, 0:288],
                                        rep[:, 288:HW], OP.mult)
                nc.vector.tensor_scalar_add(dst, dst, vcr[:])

        # ---- final conv + bias, DMA out ----
        fin = [act.tile([PD, HW], F32, name=f"fin{b}_{j}", tag=f"fin{j}")
               for j in range(NCC)]
        conv("fin", b, W["WrT"], outall, fin, bias=wrb)
        for ot in range(NCC):
            nc.sync.dma_start(d["out"][b, ot * PD:(ot + 1) * PD, :], fin[ot][:])


_CACHE = {}


def _build():
    if "nc" in _CACHE:
        return _CACHE["nc"], _CACHE["out"]
    nc = bacc.Bacc("TRN2", target_bir_lowering=False, debug=False,
                   num_devices=NCORES)
    d = {
        "x": nc.dram_tensor("x", [BPC, C, HW], F32R, kind="ExternalInput").ap(),
        "t_m_blk": nc.dram_tensor("t_m_blk", [BPC, C, TMP], F32R,
                                  kind="ExternalInput").ap(),
        "tvec": nc.dram_tensor("tvec", [BPC, C, 1], F32, kind="ExternalInput").ap(),
        "Wr_b": nc.dram_tensor("Wr_b", [C, 1], F32, kind="ExternalInput").ap(),
        "out": nc.dram_tensor("out", [BPC, C, HW], F32, kind="ExternalOutput").ap(),
    }
    for wn in ("WqT", "WkT", "WvT", "Wm1T", "WrT"):
        d[wn] = nc.dram_tensor(wn, [C, C], F32R, kind="ExternalInput").ap()
    with tile.TileContext(nc) as tc:
        with ExitStack() as ctx:
            _body(ctx, tc, d)
    nc.compile()
    _CACHE["nc"], _CACHE["out"] = nc, d["out"].tensor.name
    return nc, _CACHE["out"]


def _prep_inputs(x, t, Wk, Wq, Wt_w, Wt_b, Wm, Wv, Wr_w, Wr_b):
    f = np.float32
    x = np.asarray(x, f).reshape(B, C, HW)
    t = np.asarray(t, f)
    t_m = t @ np.asarray(Wt_w, f).T + np.asarray(Wt_b, f)
    t_m_blk = np.zeros((B, C, TMP), f)
    for h in range(NH):
        t_m_blk[:, h * CPH:(h + 1) * CPH, h] = t_m[:, h * CPH:(h + 1) * CPH]
    tvec = (t @ np.asarray(Wm, f)[:, C:].T).reshape(B, C, 1)
    com = {
        "WqT": np.ascontiguousarray(np.asarray(Wq, f).T),
        "WkT": np.ascontiguousarray(np.asarray(Wk, f).T),
        "WvT": np.ascontiguousarray(np.asarray(Wv, f).T),
        "Wm1T": np.ascontiguousarray(np.asarray(Wm, f)[:, :C].T),
        "WrT": np.ascontiguousarray(np.asarray(Wr_w, f).T),
        "Wr_b": np.asarray(Wr_b, f).reshape(C, 1),
    }
    maps = []
    for c in range(NCORES):
        sl = slice(c * BPC, (c + 1) * BPC)
        m = dict(com)
        m["x"] = np.ascontiguousarray(x[sl])
        m["t_m_blk"] = np.ascontiguousarray(t_m_blk[sl])
        m["tvec"] = np.ascontiguousarray(tvec[sl])
        maps.append(m)
    return maps


def kernel(x, t, Wk, Wq, Wt_w, Wt_b, Wm, Wv, Wr_w, Wr_b, _trace=False):
    nc, out_name = _build()
    maps = _prep_inputs(x, t, Wk, Wq, Wt_w, Wt_b, Wm, Wv, Wr_w, Wr_b)
    res = run_bass_kernel_spmd(nc, maps, core_ids=list(range(NCORES)),
                               trace=_trace)
    out = np.concatenate([res.results[c][out_name] for c in range(NCORES)],
                         axis=0).reshape(B, C, 24, 24)
    if _trace:
        kernel.last_results = res
    return out


# revision 25
# speedup vs baseline: 1.1632x; 1.0093x over previous
"""Trainium2 Bass kernel for nn_AttentionModule_50002009260608.

B=16, C=512, H=W=24 (HW=576), TF=512, NH=8, CPH=64.
Data-parallel over batch: 2 batch elements per core x 8 cores.
Weights replicated; host pre-transposes 1x1-conv weights to [c_in, c_out]
and precomputes the two tiny text matvecs (t_m, Wm2 @ t).

All heavy matmuls run as float32r (full PE rate for N>=256) accumulating
in fp32 PSUM. fp32r ISA restrictions: output must span all 128 PE columns
(M>=97) and innermost AP counts must be even -- hence hw m-tiles of
116+115*4 and the padded per-head V'T stride of 128.
"""

import numpy as np
from contextlib import ExitStack

import concourse.bacc as bacc
import concourse.bass as bass
import concourse.tile as tile
import concourse.mybir as mybir
from concourse import masks
from concourse.bass_utils import run_bass_kernel_spmd

B, C, HW, TF, NH, CPH = 16, 512, 576, 512, 8, 64
NCORES, BPC = 8, B // 8
SCALE = 1.0 / 8.0  # 1/sqrt(CPH)
F32, F32R = mybir.dt.float32, mybir.dt.float32r
AF = mybir.ActivationFunctionType
OP = mybir.AluOpType
PD = 128
NCC = C // PD                                    # 4 channel chunks
MT = [(0, 116), (116, 115), (231, 115), (346, 115), (461, 115)]  # hw m-tiles
NHALF = [(0, 288), (288, 288)]                   # softmax eviction halves
AVCH = [(0, 288), (288, 290)]                    # AV rhs chunks over es cols
CPS = 128                                        # padded per-head V'T stride
TMP = 104                                        # padded t_m_blk cols (fp32r M>=97)
ESW = HW + 2                                     # es cols: 576 + cross col + pad


def _body(ctx: ExitStack, tc, d):
    """d: DRAM APs: x[2,512,576](f32r), t_m_blk[2,512,104](f32r),
    tvec[2,512,1], WqT/WkT/WvT/Wm1T/WrT [512,512](f32r, pre-transposed
    [c_in,c_out]), Wr_b[512,1], out[2,512,576]."""
    nc = tc.nc

    wt = ctx.enter_context(tc.tile_pool(name="wt", bufs=1))
    act = ctx.enter_context(tc.tile_pool(name="act", bufs=1))
    expp = ctx.enter_context(tc.tile_pool(name="expp", bufs=1))
    ps = ctx.enter_context(tc.tile_pool(name="ps", bufs=1, space="PSUM"))

    # ---- batch-0 activations first (PE can start within ~2us), then weights,
    # all split per channel-chunk so the first conv group's deps arrive early ----
    xbts = []
    for b in range(BPC):
        xbt = act.tile([PD, NCC * HW], F32R, name=f"xb{b}", tag="xb", bufs=2)
        if b == 0:
            for j in range(NCC):
                nc.sync.dma_start(xbt[:, j * HW:(j + 1) * HW],
                                  d["x"][b, j * PD:(j + 1) * PD, :])
        xbts.append(xbt)
    W = {}
    for wn in ("WqT", "WkT", "Wm1T", "WvT", "WrT"):
        wtile = wt.tile([PD, NCC * C], F32R, name=f"{wn}_t")
        for j in range(NCC):
            nc.sync.dma_start(wtile[:, j * C:(j + 1) * C],
                              d[wn][j * PD:(j + 1) * PD, :])
        W[wn] = [wtile[:, j * C:(j + 1) * C] for j in range(NCC)]
    wrbt = wt.tile([PD, NCC], F32, name="wrbt")
    nc.sync.dma_start(wrbt[:], d["Wr_b"].rearrange("(cc p) one -> p (cc one)", p=PD))
    wrb = [wrbt[:, j:j + 1] for j in range(NCC)]
    ident = wt.tile([PD, PD], F32, name="ident")
    masks.make_identity(nc, ident[:])
    onesb = wt.tile([PD, (CPS - CPH) * NH], F32, name="onesb")
    nc.vector.memset(onesb[:], 1.0)
    # batched softmax-sum reciprocal scratch: a sub-head's 2 sums rows parked
    # at partitions 0/32, one [33, 288] reciprocal covers both halves.
    # Two tiles, alternated by sub parity, so consecutive sub-heads pipeline.
    smt2 = [wt.tile([33, 288], F32, name=f"smt{i}") for i in range(2)]
    smr2 = [wt.tile([33, 288], F32, name=f"smr{i}") for i in range(2)]
    smb2 = [wt.tile([1, 288], F32, name=f"smb{i}") for i in range(2)]
    for i in range(2):
        nc.vector.memset(smt2[i][:], 1.0)
    # persistent V'T tiles: [hw_tile, 8*128]; per head block: cols 0:64 = V_h^T,
    # cols 64:128 = 1.0 (fused softmax column sums). Ones written once.
    VT = [wt.tile([sz, NH * CPS], F32R, name=f"vt{mi}")
          for mi, (m0, sz) in enumerate(MT)]
    for mi, (m0, sz) in enumerate(MT):
        nc.vector.tensor_copy(
            VT[mi][:].rearrange("p (h c) -> p h c", h=NH)[:, :, CPH:CPS],
            onesb[0:sz, :])

    def conv(name, b, Wn, rhs, outs, bias=None):
        # outs[ot][:, n] = sum_cc Wn[cc][:, ot*128:+128].T @ rhs[cc][:, n] (+ bias)
        for ot in range(NCC):
            for (n0, nsz) in NHALF:
                p = ps.tile([PD, nsz], F32, tag="conv", bufs=2,
                            name=f"p_{name}{b}_{ot}_{n0}")
                for cc in range(NCC):
                    nc.tensor.matmul(
                        p[:], Wn[cc][:, ot * PD:(ot + 1) * PD],
                        rhs[cc][:, n0:n0 + nsz],
                        start=(cc == 0), stop=(cc == NCC - 1))
                dst = outs[ot][:, n0:n0 + nsz]
                if bias is not None:
                    nc.scalar.activation(dst, p[:], AF.Identity, bias=bias[ot])
                else:
                    nc.vector.tensor_copy(dst, p[:])

    for b in range(BPC):
        # ---- load per-batch inputs ----
        xbt = xbts[b]
        if b > 0:
            for j in range(NCC):
                nc.sync.dma_start(xbt[:, j * HW:(j + 1) * HW],
                                  d["x"][b, j * PD:(j + 1) * PD, :])
        xb = [xbt[:, j * HW:(j + 1) * HW] for j in range(NCC)]
        tvt = act.tile([PD, NCC], F32, name=f"tv{b}", tag="tv")
        nc.sync.dma_start(tvt[:],
                          d["tvec"][b].rearrange("(cc p) one -> p (cc one)", p=PD))
        tvecs = [tvt[:, j:j + 1] for j in range(NCC)]
        tmbt = act.tile([PD, NCC * TMP], F32R, name=f"tmblk{b}", tag="tmblk")
        nc.sync.dma_start(tmbt[:].rearrange("p (cc h) -> p cc h", cc=NCC),
                          d["t_m_blk"][b].rearrange("(cc p) h -> p cc h", p=PD))
        tmblk = [tmbt[:, j * TMP:(j + 1) * TMP] for j in range(NCC)]

        # ---- Q, K, vl convs ----
        Q = [act.tile([PD, HW], F32R, name=f"q{b}_{j}", tag=f"q{j}", bufs=2)
             for j in range(NCC)]
        K = [act.tile([PD, HW], F32R, name=f"k{b}_{j}", tag=f"k{j}", bufs=2)
             for j in range(NCC)]
        vl = [act.tile([PD, HW], F32R, name=f"vl{b}_{j}", tag=f"vl{j}")
              for j in range(NCC)]
        conv("q", b, W["WqT"], xb, Q)
        conv("k", b, W["WkT"], xb, K)
        conv("vl", b, W["Wm1T"], xb, vl, bias=tvecs)

        # ---- V'T: (Wv @ vl)^T into the persistent padded tiles ----
        for mi, (m0, sz) in enumerate(MT):
            p = ps.tile([sz, C], F32, tag="conv", bufs=2, name=f"p_vt{b}_{mi}")
            for cc in range(NCC):
                nc.tensor.matmul(p[:], vl[cc][:, m0:m0 + sz], W["WvT"][cc][:],
                                 start=(cc == 0), stop=(cc == NCC - 1))
            vsrc = p[:].rearrange("p (h c) -> p h c", h=NH)
            vv = VT[mi][:].rearrange("p (h c) -> p h c", h=NH)
            nc.vector.tensor_copy(vv[:, :, 0:CPH], vsrc)

        # ---- cross attention softmax (over hw, free dim) ----
        crosse = act.tile([NH, HW], F32, name=f"crosse{b}", tag="crosse")
        csum = [act.tile([NH, 1], F32, name=f"csum{b}_{i}", tag=f"csum{i}")
                for i in range(2)]
        for hi, (n0, nsz) in enumerate(NHALF):
            p = ps.tile([TMP, nsz], F32, tag="s", bufs=3, name=f"p_cl{b}_{hi}")
            for cc in range(NCC):
                nc.tensor.matmul(p[:], tmblk[cc], xb[cc][:, n0:n0 + nsz],
                                 start=(cc == 0), stop=(cc == NCC - 1))
            nc.scalar.activation(crosse[:, n0:n0 + nsz], p[0:NH, :], AF.Exp,
                                 scale=SCALE, accum_out=csum[hi][:])
        crec = act.tile([NH, 1], F32, name=f"crec{b}", tag="crec")
        nc.vector.tensor_add(crec[:], csum[0][:], csum[1][:])
        nc.vector.reciprocal(crec[:], crec[:])
        crossn = act.tile([NH, HW], F32, name=f"crossn{b}", tag="crossn")
        nc.vector.tensor_scalar_mul(crossn[:], crosse[:], crec[:])
        # transpose to [hw, 9]; identity sliced [8,9] makes col 8 a zero pad
        # (the es fold needs a finite 578th column, value irrelevant).
        crossT = [act.tile([sz, NH + 1], F32R, name=f"crossT{b}_{mi}",
                           tag=f"crossT{mi}") for mi, (m0, sz) in enumerate(MT)]
        for mi, (m0, sz) in enumerate(MT):
            pt = ps.tile([sz, NH], F32, tag="conv", bufs=2,
                         name=f"p_ct{b}_{mi}")
            nc.tensor.transpose(pt[:], crossn[:, m0:m0 + sz], ident[0:NH, 0:NH])
            nc.vector.tensor_copy(crossT[mi][0:sz, 0:NH], pt[:])
            nc.gpsimd.tensor_copy(crossT[mi][0:sz, NH:NH + 1], onesb[0:sz, 0:1])

        # ---- per-head attention, heads processed in row-group pairs ----
        outall = [act.tile([PD, HW], F32R, name=f"oa{b}_{j}", tag=f"oa{j}")
                  for j in range(NCC)]
        for hp in range(NH // 2):
            h2 = (2 * hp, 2 * hp + 1)
            # es[sub][mi]: [sz, 578]; cols 0:576 = exp(scale*S^T), col 576 =
            # crossnorm (fused cross-value column), col 577 = finite pad.
            es = [[expp.tile([sz, ESW], F32R, name=f"es{b}_{hp}_{sub}_{mi}",
                             tag=f"es{sub}_{mi}", bufs=2)
                   for mi, (m0, sz) in enumerate(MT)] for sub in range(2)]
            # S^T + exp; even/odd head matmuls adjacent -> disjoint row groups
            # (rows 0:64 vs 64:128) run concurrently in the PE array.
            for mi, (m0, sz) in enumerate(MT):
                for hi, (n0, nsz) in enumerate(NHALF):
                    for sub in range(2):
                        rr = sub * CPH
                        p = ps.tile([sz, nsz], F32, tag="s", bufs=3,
                                    name=f"p_s{b}_{hp}_{sub}_{mi}_{n0}")
                        nc.tensor.matmul(
                            p[:], K[hp][rr:rr + CPH, m0:m0 + sz],
                            Q[hp][rr:rr + CPH, n0:n0 + nsz],
                            start=True, stop=True, tile_position=(rr, 0),
                            skip_group_check=True)
                        nc.scalar.activation(es[sub][mi][:, n0:n0 + nsz], p[:],
                                             AF.Exp, scale=SCALE)
                for sub in range(2):
                    nc.gpsimd.tensor_copy(
                        es[sub][mi][:, HW:ESW],
                        crossT[mi][0:sz, h2[sub]:h2[sub] + 2])
            # AV: out rows 0:64 = V_h @ A-parts, row 64 = softmax sums;
            # av1 col 288 (global es col 576) = cross-value vector.
            for sub in range(2):
                h = h2[sub]
                pav = [ps.tile([PD, nsz], F32, tag="av", bufs=3,
                               name=f"p_av{b}_{h}_{ci}")
                       for ci, (c0, nsz) in enumerate(AVCH)]
                for mi, (m0, sz) in enumerate(MT):
                    lhs = VT[mi][:, h * CPS:(h + 1) * CPS]
                    st, sp = (mi == 0), (mi == len(MT) - 1)
                    for ci, (c0, nsz) in enumerate(AVCH):
                        nc.tensor.matmul(pav[ci][:], lhs,
                                         es[sub][mi][:, c0:c0 + nsz],
                                         start=st, stop=sp)
                # batched reciprocal of both halves' sums (rows 0 and 32)
                rr = sub * CPH
                smt = smt2[(2 * hp + sub) % 2]
                smr = smr2[(2 * hp + sub) % 2]
                smb = smb2[(2 * hp + sub) % 2]
                nc.vector.tensor_copy(smt[0:1, :], pav[0][CPH:CPH + 1, 0:288])
                nc.vector.tensor_copy(smt[32:33, :], pav[1][CPH:CPH + 1, 0:288])
                nc.vector.reciprocal(smr[:], smt[:])
                # partition_broadcast reads physical partition 0 only: move the
                # half-1 reciprocal row from partition 32 back to base 0.
                nc.vector.tensor_copy(smb[:], smr[32:33, :])
                vcr = act.tile([CPH, 1], F32, name=f"vcr{b}_{h}", tag="vcr",
                               bufs=2)
                nc.vector.tensor_copy(vcr[:], pav[1][0:CPH, 288:289])
                rep = act.tile([CPH, HW], F32, name=f"rep{b}_{h}", tag="rep",
                               bufs=2)
                nc.gpsimd.partition_broadcast(rep[:, 0:288], smr[0:1, :])
                nc.gpsimd.partition_broadcast(rep[:, 288:HW], smb[:])
                dst = outall[hp][rr:rr + CPH, :]
                nc.vector.tensor_tensor(dst[:, 0:288], pav[0][0:CPH, :],
                                        rep[:, 0:288], OP.mult)
                nc.vector.tensor_tensor(dst[:, 288:HW], pav[1][0:CPH, 0:288],
                                        rep[:, 288:HW], OP.mult)
                nc.vector.tensor_scalar_add(dst, dst, vcr[:])

        # ---- final conv + bias, DMA out ----
        fin = [act.tile([PD, HW], F32, name=f"fin{b}_{j}", tag=f"fin{j}")
               for j in range(NCC)]
        conv("fin", b, W["WrT"], outall, fin, bias=wrb)
        for ot in range(NCC):
            nc.sync.dma_start(d["out"][b, ot * PD:(ot + 1) * PD, :], fin[ot][:])


_CACHE = {}


def _build():
    if "nc" in _CACHE:
        return _CACHE["nc"], _CACHE["out"]
    nc = bacc.Bacc("TRN2", target_bir_lowering=False, debug=False,
                   num_devices=NCORES)
    d = {
        "x": nc.dram_tensor("x", [BPC, C, HW], F32R, kind="ExternalInput").ap(),
        "t_m_blk": nc.dram_tensor("t_m_blk", [BPC, C, TMP], F32R,
                                  kind="ExternalInput").ap(),
        "tvec": nc.dram_tensor("tvec", [BPC, C, 1], F32, kind="ExternalInput").ap(),
        "Wr_b": nc.dram_tensor("Wr_b", [C, 1], F32, kind="ExternalInput").ap(),
        "out": nc.dram_tensor("out", [BPC, C, HW], F32, kind="ExternalOutput").ap(),
    }
    for wn in ("WqT", "WkT", "WvT", "Wm1T", "WrT"):
        d[wn] = nc.dram_tensor(wn, [C, C], F32R, kind="ExternalInput").ap()
    with tile.TileContext(nc) as tc:
        with ExitStack() as ctx:
            _body(ctx, tc, d)
    nc.compile()
    _CACHE["nc"], _CACHE["out"] = nc, d["out"].tensor.name
    return nc, _CACHE["out"]


def _prep_inputs(x, t, Wk, Wq, Wt_w, Wt_b, Wm, Wv, Wr_w, Wr_b):
    f = np.float32
    x = np.asarray(x, f).reshape(B, C, HW)
    t = np.asarray(t, f)
    t_m = t @ np.asarray(Wt_w, f).T + np.asarray(Wt_b, f)
    t_m_blk = np.zeros((B, C, TMP), f)
    for h in range(NH):
        t_m_blk[:, h * CPH:(h + 1) * CPH, h] = t_m[:, h * CPH:(h + 1) * CPH]
    tvec = (t @ np.asarray(Wm, f)[:, C:].T).reshape(B, C, 1)
    com = {
        "WqT": np.ascontiguousarray(np.asarray(Wq, f).T),
        "WkT": np.ascontiguousarray(np.asarray(Wk, f).T),
        "WvT": np.ascontiguousarray(np.asarray(Wv, f).T),
        "Wm1T": np.ascontiguousarray(np.asarray(Wm, f)[:, :C].T),
        "WrT": np.ascontiguousarray(np.asarray(Wr_w, f).T),
        "Wr_b": np.asarray(Wr_b, f).reshape(C, 1),
    }
    maps = []
    for c in range(NCORES):
        sl = slice(c * BPC, (c + 1) * BPC)
        m = dict(com)
        m["x"] = np.ascontiguousarray(x[sl])
        m["t_m_blk"] = np.ascontiguousarray(t_m_blk[sl])
        m["tvec"] = np.ascontiguousarray(tvec[sl])
        maps.append(m)
    return maps


def kernel(x, t, Wk, Wq, Wt_w, Wt_b, Wm, Wv, Wr_w, Wr_b, _trace=False):
    nc, out_name = _build()
    maps = _prep_inputs(x, t, Wk, Wq, Wt_w, Wt_b, Wm, Wv, Wr_w, Wr_b)
    res = run_bass_kernel_spmd(nc, maps, core_ids=list(range(NCORES)),
                               trace=_trace)
    out = np.concatenate([res.results[c][out_name] for c in range(NCORES)],
                         axis=0).reshape(B, C, 24, 24)
    if _trace:
        kernel.last_results = res
    return out


# revision 27
# speedup vs baseline: 1.1994x; 1.0311x over previous
"""Trainium2 Bass kernel for nn_AttentionModule_50002009260608.

B=16, C=512, H=W=24 (HW=576), TF=512, NH=8, CPH=64.
Data-parallel over batch: 2 batch elements per core x 8 cores.
Weights replicated; host pre-transposes 1x1-conv weights to [c_in, c_out]
and precomputes the two tiny text matvecs (t_m, Wm2 @ t).

All heavy matmuls run as float32r (full PE rate for N>=256) accumulating
in fp32 PSUM. fp32r ISA restrictions: output must span all 128 PE columns
(M>=97) and innermost AP counts must be even -- hence hw m-tiles of
116+115*4 and the padded per-head V'T stride of 128.
"""

import numpy as np
from contextlib import ExitStack

import concourse.bacc as bacc
import concourse.bass as bass
import concourse.tile as tile
import concourse.mybir as mybir
from concourse import masks
from concourse.bass_utils import run_bass_kernel_spmd

B, C, HW, TF, NH, CPH = 16, 512, 576, 512, 8, 64
NCORES, BPC = 8, B // 8
SCALE = 1.0 / 8.0  # 1/sqrt(CPH)
F32, F32R = mybir.dt.float32, mybir.dt.float32r
AF = mybir.ActivationFunctionType
OP = mybir.AluOpType
PD = 128
NCC = C // PD                                    # 4 channel chunks
MT = [(0, 116), (116, 115), (231, 115), (346, 115), (461, 115)]  # hw m-tiles
NHALF = [(0, 288), (288, 288)]                   # softmax eviction halves
AVCH = [(0, 288), (288, 290)]                    # AV rhs chunks over es cols
CPS = 128                                        # padded per-head V'T stride
TMP = 104                                        # padded t_m_blk cols (fp32r M>=97)
ESW = HW + 2                                     # es cols: 576 + cross col + pad


def _body(ctx: ExitStack, tc, d):
    """d: DRAM APs: x[2,512,576](f32r), t_m_blk[2,512,104](f32r),
    tvec[2,512,1], WqT/WkT/WvT/Wm1T/WrT [512,512](f32r, pre-transposed
    [c_in,c_out]), Wr_b[512,1], out[2,512,576]."""
    nc = tc.nc

    wt = ctx.enter_context(tc.tile_pool(name="wt", bufs=1))
    act = ctx.enter_context(tc.tile_pool(name="act", bufs=1))
    expp = ctx.enter_context(tc.tile_pool(name="expp", bufs=1))
    ps = ctx.enter_context(tc.tile_pool(name="ps", bufs=1, space="PSUM"))

    # ---- batch-0 activations first (PE can start within ~2us), then weights,
    # all split per channel-chunk so the first conv group's deps arrive early ----
    xbts = []
    for b in range(BPC):
        xbt = act.tile([PD, NCC * HW], F32R, name=f"xb{b}", tag="xb", bufs=2)
        if b == 0:
            for j in range(NCC):
                nc.sync.dma_start(xbt[:, j * HW:(j + 1) * HW],
                                  d["x"][b, j * PD:(j + 1) * PD, :])
        xbts.append(xbt)
    W = {}
    for wn in ("WqT", "WkT", "Wm1T", "WvT", "WrT"):
        wtile = wt.tile([PD, NCC * C], F32R, name=f"{wn}_t")
        for j in range(NCC):
            nc.sync.dma_start(wtile[:, j * C:(j + 1) * C],
                              d[wn][j * PD:(j + 1) * PD, :])
        W[wn] = [wtile[:, j * C:(j + 1) * C] for j in range(NCC)]
    wrbt = wt.tile([PD, NCC], F32, name="wrbt")
    nc.sync.dma_start(wrbt[:], d["Wr_b"].rearrange("(cc p) one -> p (cc one)", p=PD))
    wrb = [wrbt[:, j:j + 1] for j in range(NCC)]
    ident = wt.tile([PD, PD], F32, name="ident")
    masks.make_identity(nc, ident[:])
    onesb = wt.tile([PD, (CPS - CPH) * NH], F32, name="onesb")
    nc.vector.memset(onesb[:], 1.0)
    # batched softmax-sum reciprocal scratch: a sub-head's 2 sums rows parked
    # at partitions 0/32, one [33, 288] reciprocal covers both halves.
    # Two tile sets, alternated by sub parity, so consecutive sub-heads pipeline.
    smt2 = [wt.tile([33, 288], F32, name=f"smt{i}") for i in range(2)]
    smr2 = [wt.tile([33, 288], F32, name=f"smr{i}") for i in range(2)]
    smb2 = [wt.tile([1, 288], F32, name=f"smb{i}") for i in range(2)]
    for i in range(2):
        nc.vector.memset(smt2[i][:], 1.0)
    # persistent V'T tiles: [hw_tile, 8*128]; per head block: cols 0:64 = V_h^T,
    # cols 64:128 = 1.0 (fused softmax column sums). Ones written once.
    VT = [wt.tile([sz, NH * CPS], F32R, name=f"vt{mi}")
          for mi, (m0, sz) in enumerate(MT)]
    for mi, (m0, sz) in enumerate(MT):
        nc.vector.tensor_copy(
            VT[mi][:].rearrange("p (h c) -> p h c", h=NH)[:, :, CPH:CPS],
            onesb[0:sz, :])

    def conv(name, b, Wn, rhs, outs, bias=None):
        # outs[ot][:, n] = sum_cc Wn[cc][:, ot*128:+128].T @ rhs[cc][:, n] (+ bias)
        for ot in range(NCC):
            for (n0, nsz) in NHALF:
                p = ps.tile([PD, nsz], F32, tag="conv", bufs=2,
                            name=f"p_{name}{b}_{ot}_{n0}")
                for cc in range(NCC):
                    nc.tensor.matmul(
                        p[:], Wn[cc][:, ot * PD:(ot + 1) * PD],
                        rhs[cc][:, n0:n0 + nsz],
                        start=(cc == 0), stop=(cc == NCC - 1))
                dst = outs[ot][:, n0:n0 + nsz]
                if bias is not None:
                    nc.scalar.activation(dst, p[:], AF.Identity, bias=bias[ot])
                else:
                    nc.vector.tensor_copy(dst, p[:])

    st8 = {}

    def emit_loads(b):
        xbt = xbts[b]
        if b > 0:
            for j in range(NCC):
                nc.sync.dma_start(xbt[:, j * HW:(j + 1) * HW],
                                  d["x"][b, j * PD:(j + 1) * PD, :])
        xb = [xbt[:, j * HW:(j + 1) * HW] for j in range(NCC)]
        tvt = act.tile([PD, NCC], F32, name=f"tv{b}", tag="tv")
        nc.sync.dma_start(tvt[:],
                          d["tvec"][b].rearrange("(cc p) one -> p (cc one)", p=PD))
        tmbt = act.tile([PD, NCC * TMP], F32R, name=f"tmblk{b}", tag="tmblk")
        nc.sync.dma_start(tmbt[:].rearrange("p (cc h) -> p cc h", cc=NCC),
                          d["t_m_blk"][b].rearrange("(cc p) h -> p cc h", p=PD))
        st8[b] = {
            "xb": xb,
            "tvecs": [tvt[:, j:j + 1] for j in range(NCC)],
            "tmblk": [tmbt[:, j * TMP:(j + 1) * TMP] for j in range(NCC)],
        }

    def emit_q(b):
        s = st8[b]
        s["Q"] = [act.tile([PD, HW], F32R, name=f"q{b}_{j}", tag=f"q{j}", bufs=2)
                  for j in range(NCC)]
        conv("q", b, W["WqT"], s["xb"], s["Q"])

    def emit_k(b):
        s = st8[b]
        s["K"] = [act.tile([PD, HW], F32R, name=f"k{b}_{j}", tag=f"k{j}", bufs=2)
                  for j in range(NCC)]
        conv("k", b, W["WkT"], s["xb"], s["K"])

    def emit_vl_vt(b):
        s = st8[b]
        vl = [act.tile([PD, HW], F32R, name=f"vl{b}_{j}", tag=f"vl{j}")
              for j in range(NCC)]
        conv("vl", b, W["Wm1T"], s["xb"], vl, bias=s["tvecs"])
        for mi, (m0, sz) in enumerate(MT):
            p = ps.tile([sz, C], F32, tag="conv", bufs=2, name=f"p_vt{b}_{mi}")
            for cc in range(NCC):
                nc.tensor.matmul(p[:], vl[cc][:, m0:m0 + sz], W["WvT"][cc][:],
                                 start=(cc == 0), stop=(cc == NCC - 1))
            vsrc = p[:].rearrange("p (h c) -> p h c", h=NH)
            vv = VT[mi][:].rearrange("p (h c) -> p h c", h=NH)
            nc.vector.tensor_copy(vv[:, :, 0:CPH], vsrc)

    def emit_cross(b):
        s = st8[b]
        xb, tmblk = s["xb"], s["tmblk"]
        crosse = act.tile([NH, HW], F32, name=f"crosse{b}", tag="crosse")
        csum = [act.tile([NH, 1], F32, name=f"csum{b}_{i}", tag=f"csum{i}")
                for i in range(2)]
        for hi, (n0, nsz) in enumerate(NHALF):
            p = ps.tile([TMP, nsz], F32, tag="s", bufs=3, name=f"p_cl{b}_{hi}")
            for cc in range(NCC):
                nc.tensor.matmul(p[:], tmblk[cc], xb[cc][:, n0:n0 + nsz],
                                 start=(cc == 0), stop=(cc == NCC - 1))
            nc.scalar.activation(crosse[:, n0:n0 + nsz], p[0:NH, :], AF.Exp,
                                 scale=SCALE, accum_out=csum[hi][:])
        crec = act.tile([NH, 1], F32, name=f"crec{b}", tag="crec")
        nc.vector.tensor_add(crec[:], csum[0][:], csum[1][:])
        nc.vector.reciprocal(crec[:], crec[:])
        crossn = act.tile([NH, HW], F32, name=f"crossn{b}", tag="crossn")
        nc.vector.tensor_scalar_mul(crossn[:], crosse[:], crec[:])
        crossT = [act.tile([sz, NH + 1], F32R, name=f"crossT{b}_{mi}",
                           tag=f"crossT{mi}") for mi, (m0, sz) in enumerate(MT)]
        for mi, (m0, sz) in enumerate(MT):
            pt = ps.tile([sz, NH], F32, tag="conv", bufs=2, name=f"p_ct{b}_{mi}")
            nc.tensor.transpose(pt[:], crossn[:, m0:m0 + sz], ident[0:NH, 0:NH])
            nc.vector.tensor_copy(crossT[mi][0:sz, 0:NH], pt[:])
            nc.gpsimd.tensor_copy(crossT[mi][0:sz, NH:NH + 1], onesb[0:sz, 0:1])
        s["crossT"] = crossT
        s["outall"] = [act.tile([PD, HW], F32R, name=f"oa{b}_{j}", tag=f"oa{j}")
                       for j in range(NCC)]

    def emit_pair(b, hp):
        s = st8[b]
        K, Q, crossT, outall = s["K"], s["Q"], s["crossT"], s["outall"]
        h2 = (2 * hp, 2 * hp + 1)
        es = [[expp.tile([sz, ESW], F32R, name=f"es{b}_{hp}_{sub}_{mi}",
                         tag=f"es{sub}_{mi}", bufs=2)
               for mi, (m0, sz) in enumerate(MT)] for sub in range(2)]
        for mi, (m0, sz) in enumerate(MT):
            for hi, (n0, nsz) in enumerate(NHALF):
                for sub in range(2):
                    rr = sub * CPH
                    p = ps.tile([sz, nsz], F32, tag="s", bufs=3,
                                name=f"p_s{b}_{hp}_{sub}_{mi}_{n0}")
                    nc.tensor.matmul(
                        p[:], K[hp][rr:rr + CPH, m0:m0 + sz],
                        Q[hp][rr:rr + CPH, n0:n0 + nsz],
                        start=True, stop=True, tile_position=(rr, 0),
                        skip_group_check=True)
                    nc.scalar.activation(es[sub][mi][:, n0:n0 + nsz], p[:],
                                         AF.Exp, scale=SCALE)
            for sub in range(2):
                nc.gpsimd.tensor_copy(
                    es[sub][mi][:, HW:ESW],
                    crossT[mi][0:sz, h2[sub]:h2[sub] + 2])
        for sub in range(2):
            h = h2[sub]
            pav = [ps.tile([PD, nsz], F32, tag="av", bufs=3,
                           name=f"p_av{b}_{h}_{ci}")
                   for ci, (c0, nsz) in enumerate(AVCH)]
            for mi, (m0, sz) in enumerate(MT):
                lhs = VT[mi][:, h * CPS:(h + 1) * CPS]
                st, sp = (mi == 0), (mi == len(MT) - 1)
                for ci, (c0, nsz) in enumerate(AVCH):
                    nc.tensor.matmul(pav[ci][:], lhs,
                                     es[sub][mi][:, c0:c0 + nsz],
                                     start=st, stop=sp)
            rr = sub * CPH
            smt = smt2[(2 * hp + sub) % 2]
            smr = smr2[(2 * hp + sub) % 2]
            smb = smb2[(2 * hp + sub) % 2]
            nc.vector.tensor_copy(smt[0:1, :], pav[0][CPH:CPH + 1, 0:288])
            nc.vector.tensor_copy(smt[32:33, :], pav[1][CPH:CPH + 1, 0:288])
            nc.vector.reciprocal(smr[:], smt[:])
            nc.vector.tensor_copy(smb[:], smr[32:33, :])
            vcr = act.tile([CPH, 1], F32, name=f"vcr{b}_{2 * hp + sub}",
                           tag="vcr", bufs=2)
            nc.vector.tensor_copy(vcr[:], pav[1][0:CPH, 288:289])
            rep = act.tile([CPH, HW], F32, name=f"rep{b}_{2 * hp + sub}",
                           tag="rep", bufs=2)
            nc.gpsimd.partition_broadcast(rep[:, 0:288], smr[0:1, :])
            nc.gpsimd.partition_broadcast(rep[:, 288:HW], smb[:])
            dst = outall[hp][rr:rr + CPH, :]
            nc.vector.tensor_tensor(dst[:, 0:288], pav[0][0:CPH, :],
                                    rep[:, 0:288], OP.mult)
            nc.vector.tensor_tensor(dst[:, 288:HW], pav[1][0:CPH, 0:288],
                                    rep[:, 288:HW], OP.mult)
            nc.vector.tensor_scalar_add(dst, dst, vcr[:])

    def emit_final(b):
        s = st8[b]
        fin = [act.tile([PD, HW], F32, name=f"fin{b}_{j}", tag=f"fin{j}")
               for j in range(NCC)]
        conv("fin", b, W["WrT"], s["outall"], fin, bias=wrb)
        for ot in range(NCC):
            nc.sync.dma_start(d["out"][b, ot * PD:(ot + 1) * PD, :], fin[ot][:])

    # interleave batch 1's PE-dense conv work into batch 0's head phase so the
    # tensor engine stays busy (and the HAM clock stays warm) throughout.
    emit_loads(0)
    emit_q(0)
    emit_k(0)
    emit_vl_vt(0)
    emit_cross(0)
    emit_loads(1)
    emit_pair(0, 0)
    emit_q(1)
    emit_pair(0, 1)
    emit_k(1)
    emit_pair(0, 2)
    emit_pair(0, 3)
    emit_vl_vt(1)
    emit_final(0)
    emit_cross(1)
    for hp in range(NH // 2):
        emit_pair(1, hp)
    emit_final(1)


_CACHE = {}


def _build():
    if "nc" in _CACHE:
        return _CACHE["nc"], _CACHE["out"]
    nc = bacc.Bacc("TRN2", target_bir_lowering=False, debug=False,
                   num_devices=NCORES)
    d = {
        "x": nc.dram_tensor("x", [BPC, C, HW], F32R, kind="ExternalInput").ap(),
        "t_m_blk": nc.dram_tensor("t_m_blk", [BPC, C, TMP], F32R,
                                  kind="ExternalInput").ap(),
        "tvec": nc.dram_tensor("tvec", [BPC, C, 1], F32, kind="ExternalInput").ap(),
        "Wr_b": nc.dram_tensor("Wr_b", [C, 1], F32, kind="ExternalInput").ap(),
        "out": nc.dram_tensor("out", [BPC, C, HW], F32, kind="ExternalOutput").ap(),
    }
    for wn in ("WqT", "WkT", "WvT", "Wm1T", "WrT"):
        d[wn] = nc.dram_tensor(wn, [C, C], F32R, kind="ExternalInput").ap()
    with tile.TileContext(nc) as tc:
        with ExitStack() as ctx:
            _body(ctx, tc, d)
    nc.compile()
    _CACHE["nc"], _CACHE["out"] = nc, d["out"].tensor.name
    return nc, _CACHE["out"]


def _prep_inputs(x, t, Wk, Wq, Wt_w, Wt_b, Wm, Wv, Wr_w, Wr_b):
    f = np.float32
    x = np.asarray(x, f).reshape(B, C, HW)
    t = np.asarray(t, f)
    t_m = t @ np.asarray(Wt_w, f).T + np.asarray(Wt_b, f)
    t_m_blk = np.zeros((B, C, TMP), f)
    for h in range(NH):
        t_m_blk[:, h * CPH:(h + 1) * CPH, h] = t_m[:, h * CPH:(h + 1) * CPH]
    tvec = (t @ np.asarray(Wm, f)[:, C:].T).reshape(B, C, 1)
    com = {
        "WqT": np.ascontiguousarray(np.asarray(Wq, f).T),
        "WkT": np.ascontiguousarray(np.asarray(Wk, f).T),
        "WvT": np.ascontiguousarray(np.asarray(Wv, f).T),
        "Wm1T": np.ascontiguousarray(np.asarray(Wm, f)[:, :C].T),
        "WrT": np.ascontiguousarray(np.asarray(Wr_w, f).T),
        "Wr_b": np.asarray(Wr_b, f).reshape(C, 1),
    }
    maps = []
    for c in range(NCORES):
        sl = slice(c * BPC, (c + 1) * BPC)
        m = dict(com)
        m["x"] = np.ascontiguousarray(x[sl])
        m["t_m_blk"] = np.ascontiguousarray(t_m_blk[sl])
        m["tvec"] = np.ascontiguousarray(tvec[sl])
        maps.append(m)
    return maps


def kernel(x, t, Wk, Wq, Wt_w, Wt_b, Wm, Wv, Wr_w, Wr_b, _trace=False):
    nc, out_name = _build()
    maps = _prep_inputs(x, t, Wk, Wq, Wt_w, Wt_b, Wm, Wv, Wr_w, Wr_b)
    res = run_bass_kernel_spmd(nc, maps, core_ids=list(range(NCORES)),
                               trace=_trace)
    out = np.concatenate([res.results[c][out_name] for c in range(NCORES)],
                         axis=0).reshape(B, C, 24, 24)
    if _trace:
        kernel.last_results = res
    return out


# revision 28
# speedup vs baseline: 1.2344x; 1.0292x over previous
"""Trainium2 Bass kernel for nn_AttentionModule_50002009260608.

B=16, C=512, H=W=24 (HW=576), TF=512, NH=8, CPH=64.
Data-parallel over batch: 2 batch elements per core x 8 cores.
Weights replicated; host pre-transposes 1x1-conv weights to [c_in, c_out]
and precomputes the two tiny text matvecs (t_m, Wm2 @ t).

All heavy matmuls run as float32r (full PE rate for N>=256) accumulating
in fp32 PSUM. fp32r ISA restrictions: output must span all 128 PE columns
(M>=97) and innermost AP counts must be even -- hence hw m-tiles of
116+115*4 and the padded per-head V'T stride of 128.
"""

import numpy as np
from contextlib import ExitStack

import concourse.bacc as bacc
import concourse.bass as bass
import concourse.tile as tile
import concourse.mybir as mybir
from concourse import masks
from concourse.bass_utils import run_bass_kernel_spmd

B, C, HW, TF, NH, CPH = 16, 512, 576, 512, 8, 64
NCORES, BPC = 8, B // 8
SCALE = 1.0 / 8.0  # 1/sqrt(CPH)
F32, F32R = mybir.dt.float32, mybir.dt.float32r
AF = mybir.ActivationFunctionType
OP = mybir.AluOpType
PD = 128
NCC = C // PD                                    # 4 channel chunks
MT = [(0, 116), (116, 115), (231, 115), (346, 115), (461, 115)]  # hw m-tiles
NHALF = [(0, 288), (288, 288)]                   # softmax eviction halves
AVCH = [(0, 288), (288, 290)]                    # AV rhs chunks over es cols
CPS = 128                                        # padded per-head V'T stride
TMP = 104                                        # padded t_m_blk cols (fp32r M>=97)
ESW = HW + 2                                     # es cols: 576 + cross col + pad


def _body(ctx: ExitStack, tc, d):
    """d: DRAM APs: x[2,512,576](f32r), t_m_blk[2,512,104](f32r),
    tvec[2,512,1], WqT/WkT/WvT/Wm1T/WrT [512,512](f32r, pre-transposed
    [c_in,c_out]), Wr_b[512,1], out[2,512,576]."""
    nc = tc.nc

    wt = ctx.enter_context(tc.tile_pool(name="wt", bufs=1))
    act = ctx.enter_context(tc.tile_pool(name="act", bufs=1))
    expp = ctx.enter_context(tc.tile_pool(name="expp", bufs=1))
    ps = ctx.enter_context(tc.tile_pool(name="ps", bufs=1, space="PSUM"))

    # ---- batch-0 activations first (PE can start within ~2us), then weights,
    # all split per channel-chunk so the first conv group's deps arrive early ----
    xbts = []
    for b in range(BPC):
        xbt = act.tile([PD, NCC * HW], F32R, name=f"xb{b}", tag="xb", bufs=2)
        if b == 0:
            for j in range(NCC):
                nc.sync.dma_start(xbt[:, j * HW:(j + 1) * HW],
                                  d["x"][b, j * PD:(j + 1) * PD, :])
        xbts.append(xbt)
    W = {}
    for wn in ("WqT", "WkT", "Wm1T", "WvT", "WrT"):
        wtile = wt.tile([PD, NCC * C], F32R, name=f"{wn}_t")
        for j in range(NCC):
            nc.scalar.dma_start(wtile[:, j * C:(j + 1) * C],
                                d[wn][j * PD:(j + 1) * PD, :])
        W[wn] = [wtile[:, j * C:(j + 1) * C] for j in range(NCC)]
    wrbt = wt.tile([PD, NCC], F32, name="wrbt")
    nc.sync.dma_start(wrbt[:], d["Wr_b"].rearrange("(cc p) one -> p (cc one)", p=PD))
    wrb = [wrbt[:, j:j + 1] for j in range(NCC)]
    ident = wt.tile([PD, PD], F32, name="ident")
    masks.make_identity(nc, ident[:])
    onesb = wt.tile([PD, (CPS - CPH) * NH], F32, name="onesb")
    nc.vector.memset(onesb[:], 1.0)
    # batched softmax-sum reciprocal scratch: a sub-head's 2 sums rows parked
    # at partitions 0/32, one [33, 288] reciprocal covers both halves.
    # Two tile sets, alternated by sub parity, so consecutive sub-heads pipeline.
    smt2 = [wt.tile([33, 288], F32, name=f"smt{i}") for i in range(2)]
    smr2 = [wt.tile([33, 288], F32, name=f"smr{i}") for i in range(2)]
    smb2 = [wt.tile([1, 288], F32, name=f"smb{i}") for i in range(2)]
    for i in range(2):
        nc.vector.memset(smt2[i][:], 1.0)
    # persistent V'T tiles: [hw_tile, 8*128]; per head block: cols 0:64 = V_h^T,
    # cols 64:128 = 1.0 (fused softmax column sums). Ones written once.
    VT = [wt.tile([sz, NH * CPS], F32R, name=f"vt{mi}")
          for mi, (m0, sz) in enumerate(MT)]
    for mi, (m0, sz) in enumerate(MT):
        nc.vector.tensor_copy(
            VT[mi][:].rearrange("p (h c) -> p h c", h=NH)[:, :, CPH:CPS],
            onesb[0:sz, :])

    def conv(name, b, Wn, rhs, outs, bias=None):
        # outs[ot][:, n] = sum_cc Wn[cc][:, ot*128:+128].T @ rhs[cc][:, n] (+ bias)
        for ot in range(NCC):
            for (n0, nsz) in NHALF:
                p = ps.tile([PD, nsz], F32, tag="conv", bufs=2,
                            name=f"p_{name}{b}_{ot}_{n0}")
                for cc in range(NCC):
                    nc.tensor.matmul(
                        p[:], Wn[cc][:, ot * PD:(ot + 1) * PD],
                        rhs[cc][:, n0:n0 + nsz],
                        start=(cc == 0), stop=(cc == NCC - 1))
                dst = outs[ot][:, n0:n0 + nsz]
                if bias is not None:
                    nc.scalar.activation(dst, p[:], AF.Identity, bias=bias[ot])
                else:
                    nc.vector.tensor_copy(dst, p[:])

    st8 = {}

    def emit_loads(b):
        xbt = xbts[b]
        if b > 0:
            for j in range(NCC):
                nc.sync.dma_start(xbt[:, j * HW:(j + 1) * HW],
                                  d["x"][b, j * PD:(j + 1) * PD, :])
        xb = [xbt[:, j * HW:(j + 1) * HW] for j in range(NCC)]
        tvt = act.tile([PD, NCC], F32, name=f"tv{b}", tag="tv")
        nc.sync.dma_start(tvt[:],
                          d["tvec"][b].rearrange("(cc p) one -> p (cc one)", p=PD))
        tmbt = act.tile([PD, NCC * TMP], F32R, name=f"tmblk{b}", tag="tmblk")
        nc.sync.dma_start(tmbt[:].rearrange("p (cc h) -> p cc h", cc=NCC),
                          d["t_m_blk"][b].rearrange("(cc p) h -> p cc h", p=PD))
        st8[b] = {
            "xb": xb,
            "tvecs": [tvt[:, j:j + 1] for j in range(NCC)],
            "tmblk": [tmbt[:, j * TMP:(j + 1) * TMP] for j in range(NCC)],
        }

    def emit_q(b):
        s = st8[b]
        s["Q"] = [act.tile([PD, HW], F32R, name=f"q{b}_{j}", tag=f"q{j}", bufs=2)
                  for j in range(NCC)]
        conv("q", b, W["WqT"], s["xb"], s["Q"])

    def emit_k(b):
        s = st8[b]
        s["K"] = [act.tile([PD, HW], F32R, name=f"k{b}_{j}", tag=f"k{j}", bufs=2)
                  for j in range(NCC)]
        conv("k", b, W["WkT"], s["xb"], s["K"])

    def emit_vl_vt(b):
        s = st8[b]
        vl = [act.tile([PD, HW], F32R, name=f"vl{b}_{j}", tag=f"vl{j}")
              for j in range(NCC)]
        conv("vl", b, W["Wm1T"], s["xb"], vl, bias=s["tvecs"])
        for mi, (m0, sz) in enumerate(MT):
            p = ps.tile([sz, C], F32, tag="conv", bufs=2, name=f"p_vt{b}_{mi}")
            for cc in range(NCC):
                nc.tensor.matmul(p[:], vl[cc][:, m0:m0 + sz], W["WvT"][cc][:],
                                 start=(cc == 0), stop=(cc == NCC - 1))
            vsrc = p[:].rearrange("p (h c) -> p h c", h=NH)
            vv = VT[mi][:].rearrange("p (h c) -> p h c", h=NH)
            nc.vector.tensor_copy(vv[:, :, 0:CPH], vsrc)

    def emit_cross(b):
        s = st8[b]
        xb, tmblk = s["xb"], s["tmblk"]
        crosse = act.tile([NH, HW], F32, name=f"crosse{b}", tag="crosse")
        csum = [act.tile([NH, 1], F32, name=f"csum{b}_{i}", tag=f"csum{i}")
                for i in range(2)]
        for hi, (n0, nsz) in enumerate(NHALF):
            p = ps.tile([TMP, nsz], F32, tag="s", bufs=3, name=f"p_cl{b}_{hi}")
            for cc in range(NCC):
                nc.tensor.matmul(p[:], tmblk[cc], xb[cc][:, n0:n0 + nsz],
                                 start=(cc == 0), stop=(cc == NCC - 1))
            nc.scalar.activation(crosse[:, n0:n0 + nsz], p[0:NH, :], AF.Exp,
                                 scale=SCALE, accum_out=csum[hi][:])
        crec = act.tile([NH, 1], F32, name=f"crec{b}", tag="crec")
        nc.vector.tensor_add(crec[:], csum[0][:], csum[1][:])
        nc.vector.reciprocal(crec[:], crec[:])
        crossn = act.tile([NH, HW], F32, name=f"crossn{b}", tag="crossn")
        nc.vector.tensor_scalar_mul(crossn[:], crosse[:], crec[:])
        crossT = [act.tile([sz, NH + 1], F32R, name=f"crossT{b}_{mi}",
                           tag=f"crossT{mi}") for mi, (m0, sz) in enumerate(MT)]
        for mi, (m0, sz) in enumerate(MT):
            pt = ps.tile([sz, NH], F32, tag="conv", bufs=2, name=f"p_ct{b}_{mi}")
            nc.tensor.transpose(pt[:], crossn[:, m0:m0 + sz], ident[0:NH, 0:NH])
            nc.vector.tensor_copy(crossT[mi][0:sz, 0:NH], pt[:])
            nc.gpsimd.tensor_copy(crossT[mi][0:sz, NH:NH + 1], onesb[0:sz, 0:1])
        s["crossT"] = crossT
        s["outall"] = [act.tile([PD, HW], F32R, name=f"oa{b}_{j}", tag=f"oa{j}")
                       for j in range(NCC)]

    def emit_pair(b, hp):
        s = st8[b]
        K, Q, crossT, outall = s["K"], s["Q"], s["crossT"], s["outall"]
        h2 = (2 * hp, 2 * hp + 1)
        es = [[expp.tile([sz, ESW], F32R, name=f"es{b}_{hp}_{sub}_{mi}",
                         tag=f"es{sub}_{mi}", bufs=2)
               for mi, (m0, sz) in enumerate(MT)] for sub in range(2)]
        for mi, (m0, sz) in enumerate(MT):
            for hi, (n0, nsz) in enumerate(NHALF):
                for sub in range(2):
                    rr = sub * CPH
                    p = ps.tile([sz, nsz], F32, tag="s", bufs=3,
                                name=f"p_s{b}_{hp}_{sub}_{mi}_{n0}")
                    nc.tensor.matmul(
                        p[:], K[hp][rr:rr + CPH, m0:m0 + sz],
                        Q[hp][rr:rr + CPH, n0:n0 + nsz],
                        start=True, stop=True, tile_position=(rr, 0),
                        skip_group_check=True)
                    nc.scalar.activation(es[sub][mi][:, n0:n0 + nsz], p[:],
                                         AF.Exp, scale=SCALE)
            for sub in range(2):
                nc.gpsimd.tensor_copy(
                    es[sub][mi][:, HW:ESW],
                    crossT[mi][0:sz, h2[sub]:h2[sub] + 2])
        for sub in range(2):
            h = h2[sub]
            pav = [ps.tile([PD, nsz], F32, tag="av", bufs=3,
                           name=f"p_av{b}_{h}_{ci}")
                   for ci, (c0, nsz) in enumerate(AVCH)]
            for mi, (m0, sz) in enumerate(MT):
                lhs = VT[mi][:, h * CPS:(h + 1) * CPS]
                st, sp = (mi == 0), (mi == len(MT) - 1)
                for ci, (c0, nsz) in enumerate(AVCH):
                    nc.tensor.matmul(pav[ci][:], lhs,
                                     es[sub][mi][:, c0:c0 + nsz],
                                     start=st, stop=sp)
            rr = sub * CPH
            smt = smt2[(2 * hp + sub) % 2]
            smr = smr2[(2 * hp + sub) % 2]
            smb = smb2[(2 * hp + sub) % 2]
            nc.vector.tensor_copy(smt[0:1, :], pav[0][CPH:CPH + 1, 0:288])
            nc.vector.tensor_copy(smt[32:33, :], pav[1][CPH:CPH + 1, 0:288])
            nc.vector.reciprocal(smr[:], smt[:])
            nc.vector.tensor_copy(smb[:], smr[32:33, :])
            vcr = act.tile([CPH, 1], F32, name=f"vcr{b}_{2 * hp + sub}",
                           tag="vcr", bufs=2)
            nc.vector.tensor_copy(vcr[:], pav[1][0:CPH, 288:289])
            rep = act.tile([CPH, HW], F32, name=f"rep{b}_{2 * hp + sub}",
                           tag="rep", bufs=2)
            nc.gpsimd.partition_broadcast(rep[:, 0:288], smr[0:1, :])
            nc.gpsimd.partition_broadcast(rep[:, 288:HW], smb[:])
            dst = outall[hp][rr:rr + CPH, :]
            nc.vector.tensor_tensor(dst[:, 0:288], pav[0][0:CPH, :],
                                    rep[:, 0:288], OP.mult)
            nc.vector.tensor_tensor(dst[:, 288:HW], pav[1][0:CPH, 0:288],
                                    rep[:, 288:HW], OP.mult)
            nc.vector.tensor_scalar_add(dst, dst, vcr[:])

    def emit_final(b, ots=range(NCC)):
        s = st8[b]
        if "fin" not in s:
            s["fin"] = [act.tile([PD, HW], F32, name=f"fin{b}_{j}", tag=f"fin{j}")
                        for j in range(NCC)]
        fin = s["fin"]
        for ot in ots:
            for (n0, nsz) in NHALF:
                p = ps.tile([PD, nsz], F32, tag="conv", bufs=2,
                            name=f"p_fin{b}_{ot}_{n0}")
                for cc in range(NCC):
                    nc.tensor.matmul(
                        p[:], W["WrT"][cc][:, ot * PD:(ot + 1) * PD],
                        s["outall"][cc][:, n0:n0 + nsz],
                        start=(cc == 0), stop=(cc == NCC - 1))
                nc.scalar.activation(fin[ot][:, n0:n0 + nsz], p[:], AF.Identity,
                                     bias=wrb[ot])
            nc.sync.dma_start(d["out"][b, ot * PD:(ot + 1) * PD, :], fin[ot][:])

    # interleave batch 1's PE-dense conv work into batch 0's head phase so the
    # tensor engine stays busy (and the HAM clock stays warm) throughout.
    emit_loads(0)
    emit_q(0)
    emit_k(0)
    emit_vl_vt(0)
    emit_cross(0)
    emit_loads(1)
    emit_pair(0, 0)
    emit_q(1)
    emit_pair(0, 1)
    emit_k(1)
    emit_pair(0, 2)
    emit_pair(0, 3)
    emit_vl_vt(1)
    emit_cross(1)
    emit_pair(1, 0)
    emit_final(0, [0])
    emit_pair(1, 1)
    emit_final(0, [1])
    emit_pair(1, 2)
    emit_final(0, [2])
    emit_pair(1, 3)
    emit_final(0, [3])
    emit_final(1)


_CACHE = {}


def _build():
    if "nc" in _CACHE:
        return _CACHE["nc"], _CACHE["out"]
    nc = bacc.Bacc("TRN2", target_bir_lowering=False, debug=False,
                   num_devices=NCORES)
    d = {
        "x": nc.dram_tensor("x", [BPC, C, HW], F32R, kind="ExternalInput").ap(),
        "t_m_blk": nc.dram_tensor("t_m_blk", [BPC, C, TMP], F32R,
                                  kind="ExternalInput").ap(),
        "tvec": nc.dram_tensor("tvec", [BPC, C, 1], F32, kind="ExternalInput").ap(),
        "Wr_b": nc.dram_tensor("Wr_b", [C, 1], F32, kind="ExternalInput").ap(),
        "out": nc.dram_tensor("out", [BPC, C, HW], F32, kind="ExternalOutput").ap(),
    }
    for wn in ("WqT", "WkT", "WvT", "Wm1T", "WrT"):
        d[wn] = nc.dram_tensor(wn, [C, C], F32R, kind="ExternalInput").ap()
    with tile.TileContext(nc) as tc:
        with ExitStack() as ctx:
            _body(ctx, tc, d)
    nc.compile()
    _CACHE["nc"], _CACHE["out"] = nc, d["out"].tensor.name
    return nc, _CACHE["out"]


def _prep_inputs(x, t, Wk, Wq, Wt_w, Wt_b, Wm, Wv, Wr_w, Wr_b):
    f = np.float32
    x = np.asarray(x, f).reshape(B, C, HW)
    t = np.asarray(t, f)
    t_m = t @ np.asarray(Wt_w, f).T + np.asarray(Wt_b, f)
    t_m_blk = np.zeros((B, C, TMP), f)
    for h in range(NH):
        t_m_blk[:, h * CPH:(h + 1) * CPH, h] = t_m[:, h * CPH:(h + 1) * CPH]
    tvec = (t @ np.asarray(Wm, f)[:, C:].T).reshape(B, C, 1)
    com = {
        "WqT": np.ascontiguousarray(np.asarray(Wq, f).T),
        "WkT": np.ascontiguousarray(np.asarray(Wk, f).T),
        "WvT": np.ascontiguousarray(np.asarray(Wv, f).T),
        "Wm1T": np.ascontiguousarray(np.asarray(Wm, f)[:, :C].T),
        "WrT": np.ascontiguousarray(np.asarray(Wr_w, f).T),
        "Wr_b": np.asarray(Wr_b, f).reshape(C, 1),
    }
    maps = []
    for c in range(NCORES):
        sl = slice(c * BPC, (c + 1) * BPC)
        m = dict(com)
        m["x"] = np.ascontiguousarray(x[sl])
        m["t_m_blk"] = np.ascontiguousarray(t_m_blk[sl])
        m["tvec"] = np.ascontiguousarray(tvec[sl])
        maps.append(m)
    return maps


def kernel(x, t, Wk, Wq, Wt_w, Wt_b, Wm, Wv, Wr_w, Wr_b, _trace=False):
    nc, out_name = _build()
    maps = _prep_inputs(x, t, Wk, Wq, Wt_w, Wt_b, Wm, Wv, Wr_w, Wr_b)
    res = run_bass_kernel_spmd(nc, maps, core_ids=list(range(NCORES)),
                               trace=_trace)
    out = np.concatenate([res.results[c][out_name] for c in range(NCORES)],
                         axis=0).reshape(B, C, 24, 24)
    if _trace:
        kernel.last_results = res
    return out


# revision 29
# speedup vs baseline: 1.3064x; 1.0583x over previous
"""Trainium2 Bass kernel for nn_AttentionModule_50002009260608.

B=16, C=512, H=W=24 (HW=576), TF=512, NH=8, CPH=64.
Data-parallel over batch: 2 batch elements per core x 8 cores.
Weights replicated; host pre-transposes 1x1-conv weights to [c_in, c_out]
and precomputes the two tiny text matvecs (t_m, Wm2 @ t).

All heavy matmuls run as float32r (full PE rate for N>=256) accumulating
in fp32 PSUM. fp32r ISA restrictions: output must span all 128 PE columns
(M>=97) and innermost AP counts must be even -- hence hw m-tiles of
116+115*4 and the padded per-head V'T stride of 128.
"""

import numpy as np
from contextlib import ExitStack

import concourse.bacc as bacc
import concourse.bass as bass
import concourse.tile as tile
import concourse.mybir as mybir
from concourse import masks
from concourse.bass_utils import run_bass_kernel_spmd

B, C, HW, TF, NH, CPH = 16, 512, 576, 512, 8, 64
NCORES, BPC = 8, B // 8
SCALE = 1.0 / 8.0  # 1/sqrt(CPH)
F32, F32R = mybir.dt.float32, mybir.dt.float32r
BF16 = mybir.dt.bfloat16
AF = mybir.ActivationFunctionType
OP = mybir.AluOpType
PD = 128
NCC = C // PD                                    # 4 channel chunks
MT = [(0, 116), (116, 115), (231, 115), (346, 115), (461, 115)]  # hw m-tiles
NHALF = [(0, 288), (288, 288)]                   # softmax eviction halves
AVCH = [(0, 288), (288, 290)]                    # AV rhs chunks over es cols
CPS = 128                                        # padded per-head V'T stride
TMP = 104                                        # padded t_m_blk cols (fp32r M>=97)
ESW = HW + 2                                     # es cols: 576 + cross col + pad


def _body(ctx: ExitStack, tc, d):
    """d: DRAM APs: x[2,512,576](f32r), t_m_blk[2,512,104](f32r),
    tvec[2,512,1], WqT/WkT/WvT/Wm1T/WrT [512,512](f32r, pre-transposed
    [c_in,c_out]), Wr_b[512,1], out[2,512,576]."""
    nc = tc.nc

    wt = ctx.enter_context(tc.tile_pool(name="wt", bufs=1))
    act = ctx.enter_context(tc.tile_pool(name="act", bufs=1))
    expp = ctx.enter_context(tc.tile_pool(name="expp", bufs=1))
    ps = ctx.enter_context(tc.tile_pool(name="ps", bufs=1, space="PSUM"))

    # ---- batch-0 activations first (PE can start within ~2us), then weights,
    # all split per channel-chunk so the first conv group's deps arrive early ----
    xbts = []
    for b in range(BPC):
        xbt = act.tile([PD, NCC * HW], F32R, name=f"xb{b}", tag="xb", bufs=2)
        if b == 0:
            for j in range(NCC):
                nc.sync.dma_start(xbt[:, j * HW:(j + 1) * HW],
                                  d["x"][b, j * PD:(j + 1) * PD, :])
        xbts.append(xbt)
    W = {}
    for wn in ("WqT", "WkT", "Wm1T", "WvT", "WrT"):
        wtile = wt.tile([PD, NCC * C], F32R, name=f"{wn}_t")
        for j in range(NCC):
            nc.scalar.dma_start(wtile[:, j * C:(j + 1) * C],
                                d[wn][j * PD:(j + 1) * PD, :])
        W[wn] = [wtile[:, j * C:(j + 1) * C] for j in range(NCC)]
    wrbt = wt.tile([PD, NCC], F32, name="wrbt")
    nc.sync.dma_start(wrbt[:], d["Wr_b"].rearrange("(cc p) one -> p (cc one)", p=PD))
    wrb = [wrbt[:, j:j + 1] for j in range(NCC)]
    ident = wt.tile([PD, PD], F32, name="ident")
    masks.make_identity(nc, ident[:])
    onesb = wt.tile([PD, (CPS - CPH) * NH], F32, name="onesb")
    nc.vector.memset(onesb[:], 1.0)
    # batched softmax-sum reciprocal scratch: a sub-head's 2 sums rows parked
    # at partitions 0/32, one [33, 288] reciprocal covers both halves.
    # Two tile sets, alternated by sub parity, so consecutive sub-heads pipeline.
    smt2 = [wt.tile([33, 288], F32, name=f"smt{i}") for i in range(2)]
    smr2 = [wt.tile([33, 288], F32, name=f"smr{i}") for i in range(2)]
    smb2 = [wt.tile([1, 288], F32, name=f"smb{i}") for i in range(2)]
    for i in range(2):
        nc.vector.memset(smt2[i][:], 1.0)
    # persistent V'T tiles: [hw_tile, 8*128]; per head block: cols 0:64 = V_h^T,
    # cols 64:128 = 1.0 (fused softmax column sums). Ones written once.
    VT = [wt.tile([sz, NH * CPS], BF16, name=f"vt{mi}")
          for mi, (m0, sz) in enumerate(MT)]
    for mi, (m0, sz) in enumerate(MT):
        nc.vector.tensor_copy(
            VT[mi][:].rearrange("p (h c) -> p h c", h=NH)[:, :, CPH:CPS],
            onesb[0:sz, :])

    def conv(name, b, Wn, rhs, outs, bias=None):
        # outs[ot][:, n] = sum_cc Wn[cc][:, ot*128:+128].T @ rhs[cc][:, n] (+ bias)
        for ot in range(NCC):
            for (n0, nsz) in NHALF:
                p = ps.tile([PD, nsz], F32, tag="conv", bufs=2,
                            name=f"p_{name}{b}_{ot}_{n0}")
                for cc in range(NCC):
                    nc.tensor.matmul(
                        p[:], Wn[cc][:, ot * PD:(ot + 1) * PD],
                        rhs[cc][:, n0:n0 + nsz],
                        start=(cc == 0), stop=(cc == NCC - 1))
                dst = outs[ot][:, n0:n0 + nsz]
                if bias is not None:
                    nc.scalar.activation(dst, p[:], AF.Identity, bias=bias[ot])
                else:
                    nc.vector.tensor_copy(dst, p[:])

    st8 = {}

    def emit_loads(b):
        xbt = xbts[b]
        if b > 0:
            for j in range(NCC):
                nc.sync.dma_start(xbt[:, j * HW:(j + 1) * HW],
                                  d["x"][b, j * PD:(j + 1) * PD, :])
        xb = [xbt[:, j * HW:(j + 1) * HW] for j in range(NCC)]
        tvt = act.tile([PD, NCC], F32, name=f"tv{b}", tag="tv")
        nc.sync.dma_start(tvt[:],
                          d["tvec"][b].rearrange("(cc p) one -> p (cc one)", p=PD))
        tmbt = act.tile([PD, NCC * TMP], F32R, name=f"tmblk{b}", tag="tmblk")
        nc.sync.dma_start(tmbt[:].rearrange("p (cc h) -> p cc h", cc=NCC),
                          d["t_m_blk"][b].rearrange("(cc p) h -> p cc h", p=PD))
        st8[b] = {
            "xb": xb,
            "tvecs": [tvt[:, j:j + 1] for j in range(NCC)],
            "tmblk": [tmbt[:, j * TMP:(j + 1) * TMP] for j in range(NCC)],
        }

    def emit_q(b):
        s = st8[b]
        s["Q"] = [act.tile([PD, HW], BF16, name=f"q{b}_{j}", tag=f"q{j}", bufs=2)
                  for j in range(NCC)]
        conv("q", b, W["WqT"], s["xb"], s["Q"])

    def emit_k(b):
        s = st8[b]
        s["K"] = [act.tile([PD, HW], BF16, name=f"k{b}_{j}", tag=f"k{j}", bufs=2)
                  for j in range(NCC)]
        conv("k", b, W["WkT"], s["xb"], s["K"])

    def emit_vl_vt(b):
        s = st8[b]
        vl = [act.tile([PD, HW], F32R, name=f"vl{b}_{j}", tag=f"vl{j}")
              for j in range(NCC)]
        conv("vl", b, W["Wm1T"], s["xb"], vl, bias=s["tvecs"])
        for mi, (m0, sz) in enumerate(MT):
            p = ps.tile([sz, C], F32, tag="conv", bufs=2, name=f"p_vt{b}_{mi}")
            for cc in range(NCC):
                nc.tensor.matmul(p[:], vl[cc][:, m0:m0 + sz], W["WvT"][cc][:],
                                 start=(cc == 0), stop=(cc == NCC - 1))
            vsrc = p[:].rearrange("p (h c) -> p h c", h=NH)
            vv = VT[mi][:].rearrange("p (h c) -> p h c", h=NH)
            nc.vector.tensor_copy(vv[:, :, 0:CPH], vsrc)

    def emit_cross(b):
        s = st8[b]
        xb, tmblk = s["xb"], s["tmblk"]
        crosse = act.tile([NH, HW], F32, name=f"crosse{b}", tag="crosse")
        csum = [act.tile([NH, 1], F32, name=f"csum{b}_{i}", tag=f"csum{i}")
                for i in range(2)]
        for hi, (n0, nsz) in enumerate(NHALF):
            p = ps.tile([TMP, nsz], F32, tag="s", bufs=3, name=f"p_cl{b}_{hi}")
            for cc in range(NCC):
                nc.tensor.matmul(p[:], tmblk[cc], xb[cc][:, n0:n0 + nsz],
                                 start=(cc == 0), stop=(cc == NCC - 1))
            nc.scalar.activation(crosse[:, n0:n0 + nsz], p[0:NH, :], AF.Exp,
                                 scale=SCALE, accum_out=csum[hi][:])
        crec = act.tile([NH, 1], F32, name=f"crec{b}", tag="crec")
        nc.vector.tensor_add(crec[:], csum[0][:], csum[1][:])
        nc.vector.reciprocal(crec[:], crec[:])
        crossn = act.tile([NH, HW], F32, name=f"crossn{b}", tag="crossn")
        nc.vector.tensor_scalar_mul(crossn[:], crosse[:], crec[:])
        crossT = [act.tile([sz, NH + 1], F32R, name=f"crossT{b}_{mi}",
                           tag=f"crossT{mi}") for mi, (m0, sz) in enumerate(MT)]
        for mi, (m0, sz) in enumerate(MT):
            pt = ps.tile([sz, NH], F32, tag="conv", bufs=2, name=f"p_ct{b}_{mi}")
            nc.tensor.transpose(pt[:], crossn[:, m0:m0 + sz], ident[0:NH, 0:NH])
            nc.vector.tensor_copy(crossT[mi][0:sz, 0:NH], pt[:])
            nc.gpsimd.tensor_copy(crossT[mi][0:sz, NH:NH + 1], onesb[0:sz, 0:1])
        s["crossT"] = crossT
        s["outall"] = [act.tile([PD, HW], F32R, name=f"oa{b}_{j}", tag=f"oa{j}")
                       for j in range(NCC)]

    def emit_pair(b, hp):
        s = st8[b]
        K, Q, crossT, outall = s["K"], s["Q"], s["crossT"], s["outall"]
        h2 = (2 * hp, 2 * hp + 1)
        es = [[expp.tile([sz, ESW], BF16, name=f"es{b}_{hp}_{sub}_{mi}",
                         tag=f"es{sub}_{mi}", bufs=3)
               for mi, (m0, sz) in enumerate(MT)] for sub in range(2)]
        for mi, (m0, sz) in enumerate(MT):
            for hi, (n0, nsz) in enumerate(NHALF):
                for sub in range(2):
                    rr = sub * CPH
                    p = ps.tile([sz, nsz], F32, tag="s", bufs=3,
                                name=f"p_s{b}_{hp}_{sub}_{mi}_{n0}")
                    nc.tensor.matmul(
                        p[:], K[hp][rr:rr + CPH, m0:m0 + sz],
                        Q[hp][rr:rr + CPH, n0:n0 + nsz],
                        start=True, stop=True, tile_position=(rr, 0),
                        skip_group_check=True)
                    nc.scalar.activation(es[sub][mi][:, n0:n0 + nsz], p[:],
                                         AF.Exp, scale=SCALE)
            for sub in range(2):
                nc.gpsimd.tensor_copy(
                    es[sub][mi][:, HW:ESW],
                    crossT[mi][0:sz, h2[sub]:h2[sub] + 2])
        for sub in range(2):
            h = h2[sub]
            pav = [ps.tile([PD, nsz], F32, tag="av", bufs=3,
                           name=f"p_av{b}_{h}_{ci}")
                   for ci, (c0, nsz) in enumerate(AVCH)]
            for mi, (m0, sz) in enumerate(MT):
                lhs = VT[mi][:, h * CPS:(h + 1) * CPS]
                st, sp = (mi == 0), (mi == len(MT) - 1)
                for ci, (c0, nsz) in enumerate(AVCH):
                    nc.tensor.matmul(pav[ci][:], lhs,
                                     es[sub][mi][:, c0:c0 + nsz],
                                     start=st, stop=sp)
            rr = sub * CPH
            smt = smt2[(2 * hp + sub) % 2]
            smr = smr2[(2 * hp + sub) % 2]
            smb = smb2[(2 * hp + sub) % 2]
            nc.vector.tensor_copy(smt[0:1, :], pav[0][CPH:CPH + 1, 0:288])
            nc.vector.tensor_copy(smt[32:33, :], pav[1][CPH:CPH + 1, 0:288])
            nc.vector.reciprocal(smr[:], smt[:])
            nc.vector.tensor_copy(smb[:], smr[32:33, :])
            vcr = act.tile([CPH, 1], F32, name=f"vcr{b}_{2 * hp + sub}",
                           tag="vcr", bufs=2)
            nc.vector.tensor_copy(vcr[:], pav[1][0:CPH, 288:289])
            rep = act.tile([CPH, HW], F32, name=f"rep{b}_{2 * hp + sub}",
                           tag="rep", bufs=2)
            nc.gpsimd.partition_broadcast(rep[:, 0:288], smr[0:1, :])
            nc.gpsimd.partition_broadcast(rep[:, 288:HW], smb[:])
            dst = outall[hp][rr:rr + CPH, :]
            nc.vector.tensor_tensor(dst[:, 0:288], pav[0][0:CPH, :],
                                    rep[:, 0:288], OP.mult)
            nc.vector.tensor_tensor(dst[:, 288:HW], pav[1][0:CPH, 0:288],
                                    rep[:, 288:HW], OP.mult)
            nc.vector.tensor_scalar_add(dst, dst, vcr[:])

    def emit_final(b, ots=range(NCC)):
        s = st8[b]
        if "fin" not in s:
            s["fin"] = [act.tile([PD, HW], F32, name=f"fin{b}_{j}", tag=f"fin{j}")
                        for j in range(NCC)]
        fin = s["fin"]
        for ot in ots:
            for (n0, nsz) in NHALF:
                p = ps.tile([PD, nsz], F32, tag="conv", bufs=2,
                            name=f"p_fin{b}_{ot}_{n0}")
                for cc in range(NCC):
                    nc.tensor.matmul(
                        p[:], W["WrT"][cc][:, ot * PD:(ot + 1) * PD],
                        s["outall"][cc][:, n0:n0 + nsz],
                        start=(cc == 0), stop=(cc == NCC - 1))
                nc.scalar.activation(fin[ot][:, n0:n0 + nsz], p[:], AF.Identity,
                                     bias=wrb[ot])
            nc.sync.dma_start(d["out"][b, ot * PD:(ot + 1) * PD, :], fin[ot][:])

    # interleave batch 1's PE-dense conv work into batch 0's head phase so the
    # tensor engine stays busy (and the HAM clock stays warm) throughout.
    emit_loads(0)
    emit_q(0)
    emit_k(0)
    emit_vl_vt(0)
    emit_cross(0)
    emit_loads(1)
    emit_pair(0, 0)
    emit_q(1)
    emit_pair(0, 1)
    emit_k(1)
    emit_pair(0, 2)
    emit_pair(0, 3)
    emit_vl_vt(1)
    emit_cross(1)
    emit_pair(1, 0)
    emit_final(0, [0])
    emit_pair(1, 1)
    emit_final(0, [1])
    emit_pair(1, 2)
    emit_final(0, [2])
    emit_pair(1, 3)
    emit_final(0, [3])
    emit_final(1)


_CACHE = {}


def _build():
    if "nc" in _CACHE:
        return _CACHE["nc"], _CACHE["out"]
    nc = bacc.Bacc("TRN2", target_bir_lowering=False, debug=False,
                   num_devices=NCORES)
    d = {
        "x": nc.dram_tensor("x", [BPC, C, HW], F32R, kind="ExternalInput").ap(),
        "t_m_blk": nc.dram_tensor("t_m_blk", [BPC, C, TMP], F32R,
                                  kind="ExternalInput").ap(),
        "tvec": nc.dram_tensor("tvec", [BPC, C, 1], F32, kind="ExternalInput").ap(),
        "Wr_b": nc.dram_tensor("Wr_b", [C, 1], F32, kind="ExternalInput").ap(),
        "out": nc.dram_tensor("out", [BPC, C, HW], F32, kind="ExternalOutput").ap(),
    }
    for wn in ("WqT", "WkT", "WvT", "Wm1T", "WrT"):
        d[wn] = nc.dram_tensor(wn, [C, C], F32R, kind="ExternalInput").ap()
    with tile.TileContext(nc) as tc:
        with ExitStack() as ctx:
            _body(ctx, tc, d)
    nc.compile()
    _CACHE["nc"], _CACHE["out"] = nc, d["out"].tensor.name
    return nc, _CACHE["out"]


def _prep_inputs(x, t, Wk, Wq, Wt_w, Wt_b, Wm, Wv, Wr_w, Wr_b):
    f = np.float32
    x = np.asarray(x, f).reshape(B, C, HW)
    t = np.asarray(t, f)
    t_m = t @ np.asarray(Wt_w, f).T + np.asarray(Wt_b, f)
    t_m_blk = np.zeros((B, C, TMP), f)
    for h in range(NH):
        t_m_blk[:, h * CPH:(h + 1) * CPH, h] = t_m[:, h * CPH:(h + 1) * CPH]
    tvec = (t @ np.asarray(Wm, f)[:, C:].T).reshape(B, C, 1)
    com = {
        "WqT": np.ascontiguousarray(np.asarray(Wq, f).T),
        "WkT": np.ascontiguousarray(np.asarray(Wk, f).T),
        "WvT": np.ascontiguousarray(np.asarray(Wv, f).T),
        "Wm1T": np.ascontiguousarray(np.asarray(Wm, f)[:, :C].T),
        "WrT": np.ascontiguousarray(np.asarray(Wr_w, f).T),
        "Wr_b": np.asarray(Wr_b, f).reshape(C, 1),
    }
    maps = []
    for c in range(NCORES):
        sl = slice(c * BPC, (c + 1) * BPC)
        m = dict(com)
        m["x"] = np.ascontiguousarray(x[sl])
        m["t_m_blk"] = np.ascontiguousarray(t_m_blk[sl])
        m["tvec"] = np.ascontiguousarray(tvec[sl])
        maps.append(m)
    return maps


def kernel(x, t, Wk, Wq, Wt_w, Wt_b, Wm, Wv, Wr_w, Wr_b, _trace=False):
    nc, out_name = _build()
    maps = _prep_inputs(x, t, Wk, Wq, Wt_w, Wt_b, Wm, Wv, Wr_w, Wr_b)
    res = run_bass_kernel_spmd(nc, maps, core_ids=list(range(NCORES)),
                               trace=_trace)
    out = np.concatenate([res.results[c][out_name] for c in range(NCORES)],
                         axis=0).reshape(B, C, 24, 24)
    if _trace:
        kernel.last_results = res
    return out


# revision 30
# speedup vs baseline: 1.4583x; 1.1163x over previous
"""Trainium2 Bass kernel for nn_AttentionModule_50002009260608.

B=16, C=512, H=W=24 (HW=576), TF=512, NH=8, CPH=64.
Data-parallel over batch: 2 batch elements per core x 8 cores.
Weights replicated; host pre-transposes 1x1-conv weights to [c_in, c_out]
and precomputes the two tiny text matvecs (t_m, Wm2 @ t).

All heavy matmuls run as float32r (full PE rate for N>=256) accumulating
in fp32 PSUM. fp32r ISA restrictions: output must span all 128 PE columns
(M>=97) and innermost AP counts must be even -- hence hw m-tiles of
116+115*4 and the padded per-head V'T stride of 128.
"""

import ml_dtypes
import numpy as np
from contextlib import ExitStack

import concourse.bacc as bacc
import concourse.bass as bass
import concourse.tile as tile
import concourse.mybir as mybir
from concourse import masks
from concourse.bass_utils import run_bass_kernel_spmd

B, C, HW, TF, NH, CPH = 16, 512, 576, 512, 8, 64
NCORES, BPC = 8, B // 8
SCALE = 1.0 / 8.0  # 1/sqrt(CPH)
F32, F32R = mybir.dt.float32, mybir.dt.float32r
BF16 = mybir.dt.bfloat16
AF = mybir.ActivationFunctionType
OP = mybir.AluOpType
PD = 128
NCC = C // PD                                    # 4 channel chunks
MT = [(0, 116), (116, 115), (231, 115), (346, 115), (461, 115)]  # hw m-tiles
NHALF = [(0, 288), (288, 288)]                   # softmax eviction halves
AVCH = [(0, 288), (288, 290)]                    # AV rhs chunks over es cols
CPS = 128                                        # padded per-head V'T stride
TMP = 104                                        # padded t_m_blk cols (fp32r M>=97)
ESW = HW + 2                                     # es cols: 576 + cross col + pad


def _body(ctx: ExitStack, tc, d):
    """d: DRAM APs: x[2,512,576](f32r), t_m_blk[2,512,104](f32r),
    tvec[2,512,1], WqT/WkT/WvT/Wm1T/WrT [512,512](f32r, pre-transposed
    [c_in,c_out]), Wr_b[512,1], out[2,512,576]."""
    nc = tc.nc

    wt = ctx.enter_context(tc.tile_pool(name="wt", bufs=1))
    act = ctx.enter_context(tc.tile_pool(name="act", bufs=1))
    expp = ctx.enter_context(tc.tile_pool(name="expp", bufs=1))
    ps = ctx.enter_context(tc.tile_pool(name="ps", bufs=1, space="PSUM"))

    # ---- batch-0 activations first (PE can start within ~2us), then weights,
    # all split per channel-chunk so the first conv group's deps arrive early ----
    xbts = []
    for b in range(BPC):
        xbt = act.tile([PD, NCC * HW], BF16, name=f"xb{b}", tag="xb", bufs=2)
        if b == 0:
            for j in range(NCC):
                nc.sync.dma_start(xbt[:, j * HW:(j + 1) * HW],
                                  d["x"][b, j * PD:(j + 1) * PD, :])
        xbts.append(xbt)
    W = {}
    for wn in ("WqT", "WkT", "Wm1T", "WvT", "WrT"):
        wtile = wt.tile([PD, NCC * C], BF16, name=f"{wn}_t")
        for j in range(NCC):
            nc.scalar.dma_start(wtile[:, j * C:(j + 1) * C],
                                d[wn][j * PD:(j + 1) * PD, :])
        W[wn] = [wtile[:, j * C:(j + 1) * C] for j in range(NCC)]
    wrbt = wt.tile([PD, NCC], F32, name="wrbt")
    nc.sync.dma_start(wrbt[:], d["Wr_b"].rearrange("(cc p) one -> p (cc one)", p=PD))
    wrb = [wrbt[:, j:j + 1] for j in range(NCC)]
    ident = wt.tile([PD, PD], F32, name="ident")
    masks.make_identity(nc, ident[:])
    onesb = wt.tile([PD, (CPS - CPH) * NH], F32, name="onesb")
    nc.vector.memset(onesb[:], 1.0)
    # batched softmax-sum reciprocal scratch: a sub-head's 2 sums rows parked
    # at partitions 0/32, one [33, 288] reciprocal covers both halves.
    # Two tile sets, alternated by sub parity, so consecutive sub-heads pipeline.
    smt2 = [wt.tile([33, 288], F32, name=f"smt{i}") for i in range(2)]
    smr2 = [wt.tile([33, 288], F32, name=f"smr{i}") for i in range(2)]
    smb2 = [wt.tile([1, 288], F32, name=f"smb{i}") for i in range(2)]
    for i in range(2):
        nc.vector.memset(smt2[i][:], 1.0)
    # persistent V'T tiles: [hw_tile, 8*128]; per head block: cols 0:64 = V_h^T,
    # cols 64:128 = 1.0 (fused softmax column sums). Ones written once.
    VT = [wt.tile([sz, NH * CPS], BF16, name=f"vt{mi}")
          for mi, (m0, sz) in enumerate(MT)]
    for mi, (m0, sz) in enumerate(MT):
        nc.vector.tensor_copy(
            VT[mi][:].rearrange("p (h c) -> p h c", h=NH)[:, :, CPH:CPS],
            onesb[0:sz, :])

    def conv(name, b, Wn, rhs, outs, bias=None):
        # outs[ot][:, n] = sum_cc Wn[cc][:, ot*128:+128].T @ rhs[cc][:, n] (+ bias)
        for ot in range(NCC):
            for (n0, nsz) in NHALF:
                p = ps.tile([PD, nsz], F32, tag="conv", bufs=2,
                            name=f"p_{name}{b}_{ot}_{n0}")
                for cc in range(NCC):
                    nc.tensor.matmul(
                        p[:], Wn[cc][:, ot * PD:(ot + 1) * PD],
                        rhs[cc][:, n0:n0 + nsz],
                        start=(cc == 0), stop=(cc == NCC - 1))
                dst = outs[ot][:, n0:n0 + nsz]
                if bias is not None:
                    nc.scalar.activation(dst, p[:], AF.Identity, bias=bias[ot])
                else:
                    nc.vector.tensor_copy(dst, p[:])

    st8 = {}

    def emit_loads(b):
        xbt = xbts[b]
        if b > 0:
            for j in range(NCC):
                nc.sync.dma_start(xbt[:, j * HW:(j + 1) * HW],
                                  d["x"][b, j * PD:(j + 1) * PD, :])
        xb = [xbt[:, j * HW:(j + 1) * HW] for j in range(NCC)]
        tvt = act.tile([PD, NCC], F32, name=f"tv{b}", tag="tv")
        nc.sync.dma_start(tvt[:],
                          d["tvec"][b].rearrange("(cc p) one -> p (cc one)", p=PD))
        tmbt = act.tile([PD, NCC * TMP], BF16, name=f"tmblk{b}", tag="tmblk")
        nc.sync.dma_start(tmbt[:].rearrange("p (cc h) -> p cc h", cc=NCC),
                          d["t_m_blk"][b].rearrange("(cc p) h -> p cc h", p=PD))
        st8[b] = {
            "xb": xb,
            "tvecs": [tvt[:, j:j + 1] for j in range(NCC)],
            "tmblk": [tmbt[:, j * TMP:(j + 1) * TMP] for j in range(NCC)],
        }

    def emit_q(b):
        s = st8[b]
        s["Q"] = [act.tile([PD, HW], BF16, name=f"q{b}_{j}", tag=f"q{j}", bufs=2)
                  for j in range(NCC)]
        conv("q", b, W["WqT"], s["xb"], s["Q"])

    def emit_k(b):
        s = st8[b]
        s["K"] = [act.tile([PD, HW], BF16, name=f"k{b}_{j}", tag=f"k{j}", bufs=2)
                  for j in range(NCC)]
        conv("k", b, W["WkT"], s["xb"], s["K"])

    def emit_vl_vt(b):
        s = st8[b]
        vl = [act.tile([PD, HW], BF16, name=f"vl{b}_{j}", tag=f"vl{j}")
              for j in range(NCC)]
        conv("vl", b, W["Wm1T"], s["xb"], vl, bias=s["tvecs"])
        for mi, (m0, sz) in enumerate(MT):
            p = ps.tile([sz, C], F32, tag="conv", bufs=2, name=f"p_vt{b}_{mi}")
            for cc in range(NCC):
                nc.tensor.matmul(p[:], vl[cc][:, m0:m0 + sz], W["WvT"][cc][:],
                                 start=(cc == 0), stop=(cc == NCC - 1))
            vsrc = p[:].rearrange("p (h c) -> p h c", h=NH)
            vv = VT[mi][:].rearrange("p (h c) -> p h c", h=NH)
            nc.vector.tensor_copy(vv[:, :, 0:CPH], vsrc)

    def emit_cross(b):
        s = st8[b]
        xb, tmblk = s["xb"], s["tmblk"]
        crosse = act.tile([NH, HW], F32, name=f"crosse{b}", tag="crosse")
        csum = [act.tile([NH, 1], F32, name=f"csum{b}_{i}", tag=f"csum{i}")
                for i in range(2)]
        for hi, (n0, nsz) in enumerate(NHALF):
            p = ps.tile([TMP, nsz], F32, tag="s", bufs=3, name=f"p_cl{b}_{hi}")
            for cc in range(NCC):
                nc.tensor.matmul(p[:], tmblk[cc], xb[cc][:, n0:n0 + nsz],
                                 start=(cc == 0), stop=(cc == NCC - 1))
            nc.scalar.activation(crosse[:, n0:n0 + nsz], p[0:NH, :], AF.Exp,
                                 scale=SCALE, accum_out=csum[hi][:])
        crec = act.tile([NH, 1], F32, name=f"crec{b}", tag="crec")
        nc.vector.tensor_add(crec[:], csum[0][:], csum[1][:])
        nc.vector.reciprocal(crec[:], crec[:])
        crossn = act.tile([NH, HW], F32, name=f"crossn{b}", tag="crossn")
        nc.vector.tensor_scalar_mul(crossn[:], crosse[:], crec[:])
        crossT = [act.tile([sz, NH + 1], BF16, name=f"crossT{b}_{mi}",
                           tag=f"crossT{mi}") for mi, (m0, sz) in enumerate(MT)]
        for mi, (m0, sz) in enumerate(MT):
            pt = ps.tile([sz, NH], F32, tag="conv", bufs=2, name=f"p_ct{b}_{mi}")
            nc.tensor.transpose(pt[:], crossn[:, m0:m0 + sz], ident[0:NH, 0:NH])
            nc.vector.tensor_copy(crossT[mi][0:sz, 0:NH], pt[:])
            nc.gpsimd.tensor_copy(crossT[mi][0:sz, NH:NH + 1], onesb[0:sz, 0:1])
        s["crossT"] = crossT
        s["outall"] = [act.tile([PD, HW], BF16, name=f"oa{b}_{j}", tag=f"oa{j}")
                       for j in range(NCC)]

    def emit_pair(b, hp):
        s = st8[b]
        K, Q, crossT, outall = s["K"], s["Q"], s["crossT"], s["outall"]
        h2 = (2 * hp, 2 * hp + 1)
        es = [[expp.tile([sz, ESW], BF16, name=f"es{b}_{hp}_{sub}_{mi}",
                         tag=f"es{sub}_{mi}", bufs=3)
               for mi, (m0, sz) in enumerate(MT)] for sub in range(2)]
        for mi, (m0, sz) in enumerate(MT):
            for hi, (n0, nsz) in enumerate(NHALF):
                for sub in range(2):
                    rr = sub * CPH
                    p = ps.tile([sz, nsz], F32, tag="s", bufs=3,
                                name=f"p_s{b}_{hp}_{sub}_{mi}_{n0}")
                    nc.tensor.matmul(
                        p[:], K[hp][rr:rr + CPH, m0:m0 + sz],
                        Q[hp][rr:rr + CPH, n0:n0 + nsz],
                        start=True, stop=True, tile_position=(rr, 0),
                        skip_group_check=True)
                    nc.scalar.activation(es[sub][mi][:, n0:n0 + nsz], p[:],
                                         AF.Exp, scale=SCALE)
            for sub in range(2):
                nc.gpsimd.tensor_copy(
                    es[sub][mi][:, HW:ESW],
                    crossT[mi][0:sz, h2[sub]:h2[sub] + 2])
        for sub in range(2):
            h = h2[sub]
            pav = [ps.tile([PD, nsz], F32, tag="av", bufs=3,
                           name=f"p_av{b}_{h}_{ci}")
                   for ci, (c0, nsz) in enumerate(AVCH)]
            for mi, (m0, sz) in enumerate(MT):
                lhs = VT[mi][:, h * CPS:(h + 1) * CPS]
                st, sp = (mi == 0), (mi == len(MT) - 1)
                for ci, (c0, nsz) in enumerate(AVCH):
                    nc.tensor.matmul(pav[ci][:], lhs,
                                     es[sub][mi][:, c0:c0 + nsz],
                                     start=st, stop=sp)
            rr = sub * CPH
            smt = smt2[(2 * hp + sub) % 2]
            smr = smr2[(2 * hp + sub) % 2]
            smb = smb2[(2 * hp + sub) % 2]
            nc.vector.tensor_copy(smt[0:1, :], pav[0][CPH:CPH + 1, 0:288])
            nc.vector.tensor_copy(smt[32:33, :], pav[1][CPH:CPH + 1, 0:288])
            nc.vector.reciprocal(smr[:], smt[:])
            nc.vector.tensor_copy(smb[:], smr[32:33, :])
            vcr = act.tile([CPH, 1], F32, name=f"vcr{b}_{2 * hp + sub}",
                           tag="vcr", bufs=2)
            nc.vector.tensor_copy(vcr[:], pav[1][0:CPH, 288:289])
            rep = act.tile([CPH, HW], F32, name=f"rep{b}_{2 * hp + sub}",
                           tag="rep", bufs=2)
            nc.gpsimd.partition_broadcast(rep[:, 0:288], smr[0:1, :])
            nc.gpsimd.partition_broadcast(rep[:, 288:HW], smb[:])
            dst = outall[hp][rr:rr + CPH, :]
            nc.vector.tensor_tensor(dst[:, 0:288], pav[0][0:CPH, :],
                                    rep[:, 0:288], OP.mult)
            nc.vector.tensor_tensor(dst[:, 288:HW], pav[1][0:CPH, 0:288],
                                    rep[:, 288:HW], OP.mult)
            nc.vector.tensor_scalar_add(dst, dst, vcr[:])

    def emit_final(b, ots=range(NCC)):
        s = st8[b]
        if "fin" not in s:
            s["fin"] = [act.tile([PD, HW], F32, name=f"fin{b}_{j}", tag=f"fin{j}")
                        for j in range(NCC)]
        fin = s["fin"]
        for ot in ots:
            for (n0, nsz) in NHALF:
                p = ps.tile([PD, nsz], F32, tag="conv", bufs=2,
                            name=f"p_fin{b}_{ot}_{n0}")
                for cc in range(NCC):
                    nc.tensor.matmul(
                        p[:], W["WrT"][cc][:, ot * PD:(ot + 1) * PD],
                        s["outall"][cc][:, n0:n0 + nsz],
                        start=(cc == 0), stop=(cc == NCC - 1))
                nc.scalar.activation(fin[ot][:, n0:n0 + nsz], p[:], AF.Identity,
                                     bias=wrb[ot])
            nc.sync.dma_start(d["out"][b, ot * PD:(ot + 1) * PD, :], fin[ot][:])

    # interleave batch 1's PE-dense conv work into batch 0's head phase so the
    # tensor engine stays busy (and the HAM clock stays warm) throughout.
    emit_loads(0)
    emit_q(0)
    emit_k(0)
    emit_vl_vt(0)
    emit_cross(0)
    emit_loads(1)
    emit_pair(0, 0)
    emit_q(1)
    emit_pair(0, 1)
    emit_k(1)
    emit_pair(0, 2)
    emit_pair(0, 3)
    emit_vl_vt(1)
    emit_cross(1)
    emit_pair(1, 0)
    emit_final(0, [0])
    emit_pair(1, 1)
    emit_final(0, [1])
    emit_pair(1, 2)
    emit_final(0, [2])
    emit_pair(1, 3)
    emit_final(0, [3])
    emit_final(1)


_CACHE = {}


def _build():
    if "nc" in _CACHE:
        return _CACHE["nc"], _CACHE["out"]
    nc = bacc.Bacc("TRN2", target_bir_lowering=False, debug=False,
                   num_devices=NCORES)
    d = {
        "x": nc.dram_tensor("x", [BPC, C, HW], BF16, kind="ExternalInput").ap(),
        "t_m_blk": nc.dram_tensor("t_m_blk", [BPC, C, TMP], BF16,
                                  kind="ExternalInput").ap(),
        "tvec": nc.dram_tensor("tvec", [BPC, C, 1], F32, kind="ExternalInput").ap(),
        "Wr_b": nc.dram_tensor("Wr_b", [C, 1], F32, kind="ExternalInput").ap(),
        "out": nc.dram_tensor("out", [BPC, C, HW], F32, kind="ExternalOutput").ap(),
    }
    for wn in ("WqT", "WkT", "WvT", "Wm1T", "WrT"):
        d[wn] = nc.dram_tensor(wn, [C, C], BF16, kind="ExternalInput").ap()
    with tile.TileContext(nc) as tc:
        with ExitStack() as ctx:
            _body(ctx, tc, d)
    nc.compile()
    _CACHE["nc"], _CACHE["out"] = nc, d["out"].tensor.name
    return nc, _CACHE["out"]


def _prep_inputs(x, t, Wk, Wq, Wt_w, Wt_b, Wm, Wv, Wr_w, Wr_b):
    f = np.float32
    x = np.asarray(x, f).reshape(B, C, HW)
    t = np.asarray(t, f)
    t_m = t @ np.asarray(Wt_w, f).T + np.asarray(Wt_b, f)
    t_m_blk = np.zeros((B, C, TMP), f)
    for h in range(NH):
        t_m_blk[:, h * CPH:(h + 1) * CPH, h] = t_m[:, h * CPH:(h + 1) * CPH]
    tvec = (t @ np.asarray(Wm, f)[:, C:].T).reshape(B, C, 1)
    bf = ml_dtypes.bfloat16
    com = {
        "WqT": np.ascontiguousarray(np.asarray(Wq, f).T).astype(bf),
        "WkT": np.ascontiguousarray(np.asarray(Wk, f).T).astype(bf),
        "WvT": np.ascontiguousarray(np.asarray(Wv, f).T).astype(bf),
        "Wm1T": np.ascontiguousarray(np.asarray(Wm, f)[:, :C].T).astype(bf),
        "WrT": np.ascontiguousarray(np.asarray(Wr_w, f).T).astype(bf),
        "Wr_b": np.asarray(Wr_b, f).reshape(C, 1),
    }
    maps = []
    for c in range(NCORES):
        sl = slice(c * BPC, (c + 1) * BPC)
        m = dict(com)
        m["x"] = np.ascontiguousarray(x[sl]).astype(bf)
        m["t_m_blk"] = np.ascontiguousarray(t_m_blk[sl]).astype(bf)
        m["tvec"] = np.ascontiguousarray(tvec[sl])
        maps.append(m)
    return maps


def kernel(x, t, Wk, Wq, Wt_w, Wt_b, Wm, Wv, Wr_w, Wr_b, _trace=False):
    nc, out_name = _build()
    maps = _prep_inputs(x, t, Wk, Wq, Wt_w, Wt_b, Wm, Wv, Wr_w, Wr_b)
    res = run_bass_kernel_spmd(nc, maps, core_ids=list(range(NCORES)),
                               trace=_trace)
    out = np.concatenate([res.results[c][out_name] for c in range(NCORES)],
                         axis=0).reshape(B, C, 24, 24)
    if _trace:
        kernel.last_results = res
    return out
